# revision 25
# baseline (speedup 1.0000x reference)
"""Trainium2 Bass kernel for nn_MixedOp (gnn_message_passing).

Strategy: data-parallel over batch (B=8 -> 1 sample/core). Everything on
device except zero-FLOP host marshaling (reshapes / weight transposes /
diagonal extraction; adj_w is the identity so diag(adj_w) == 1).

Math restructuring (exact up to FP reassociation):
- softmax(5A)@Z = (1/rowsum(exp(5S))) * (exp(5S) @ Z) with S = nf@nf^T
  symmetric -> exp(5S) symmetric -> its tiles usable as matmul lhsT
  directly (no attention-matrix transpose).
- diff_prop: y = s*(xW+b) - rec*(E@(xW)) with E = exp(5S) diag-zeroed,
  rec = 1/max(rowsum E, eps), s = rowsum(E)*rec.
- node_att: A@wA = nf @ (nf^T @ wA): two matvecs, A never materialized.
- temp_conv: one-hot(argmax) per 32-node frame; gather + depthwise conv
  folded into PSUM-accumulated matmuls: y_to = sum_k OH_{to+k}^T-slice @
  (X * w_k)_{to+k}-rows.
"""

import sys

for _p in ("/opt/trn_rl_repo",):
    if _p not in sys.path:
        sys.path.insert(0, _p)

import numpy as np

import concourse.bass as bass
import concourse.tile as tile
from concourse import mybir
from concourse.masks import make_identity

F32 = mybir.dt.float32
AF = mybir.ActivationFunctionType
ALU = mybir.AluOpType
AX = mybir.AxisListType

P = 128
N = 512          # nodes per sample (T*NODES)
C = 512          # channels
NT = N // P      # node tiles
CT = C // P      # channel tiles
T = 16           # frames
NPF = 32         # nodes per frame
M = 49           # global nodes per frame
GN = T * M       # 784 global rows
GT_TILES = (GN + P - 1) // P  # 7
KW = 7           # conv taps
TO = T - KW + 1  # 10 conv outputs
POSD = 9
EPS = 1e-12


def split_multiwaits(nc):
    """This walrus build accepts at most ONE sync-wait per instruction;
    Tile emits several. Hoist all but the last onto same-engine NOPs
    placed immediately before the instruction (same blocking semantics;
    each engine's program order is a topological order of the dep graph,
    so the hoisted waits cannot deadlock)."""
    n = 0
    for fn in nc.m.functions:
        for bb in fn.blocks:
            new_list = []
            for inst in bb.instructions:
                si = inst.sync_info
                if si is not None and si.on_wait and len(si.on_wait) > 1:
                    waits = list(si.on_wait)
                    for w in waits[:-1]:
                        nop = mybir.InstNoOp(
                            name=f"I-splitw-{n}",
                            engine=inst.engine,
                            ins=[],
                            outs=[],
                            sync_info=mybir.SyncInfo(on_wait=[w], on_update=[]),
                        )
                        n += 1
                        nc.register_instruction(nop)
                        new_list.append(nop)
                    inst.sync_info = mybir.SyncInfo(
                        on_wait=[waits[-1]], on_update=si.on_update
                    )
                new_list.append(inst)
            bb.instructions[:] = new_list
    return n


def _bcast_row(nc, pool, dram_ap, length, tag="brow"):
    """DMA a [length] DRAM vector to SBUF broadcast across 128 partitions."""
    t = pool.tile([P, length], F32, tag=tag, name=tag)
    src = bass.AP(tensor=dram_ap.tensor, offset=dram_ap.offset,
                  ap=[[0, P], [1, length]])
    nc.sync.dma_start(out=t, in_=src)
    return t


F32R = mybir.dt.float32r


class Builder:
    def __init__(self, nc):
        self.nc = nc
        import os
        self.use_f32r = os.environ.get("KF32R", "1") == "1"
        self.use_lrelu_act = os.environ.get("KLRELU", "1") == "1"

    def mm(self, ps, lhsT, rhs, **kw):
        """matmul, using float32r (full-rate fp32 streaming, ~13-bit
        mantissa) for wide moving operands. Operand tiles must have been
        written through a float32r-rounding producer."""
        if (self.use_f32r and lhsT.dtype == F32 and rhs.dtype == F32
                and rhs.free_size() >= 256):
            lhsT = lhsT.bitcast(F32R)
            rhs = rhs.bitcast(F32R)
        self.nc.tensor.matmul(ps, lhsT, rhs, **kw)

    def rcopy(self, out, in_):
        """ACT copy that rounds to f32r (matmul-operand producer)."""
        o = out.bitcast(F32R) if self.use_f32r else out
        self.nc.scalar.copy(out=o, in_=in_)

    _rr = 0

    def copy_any(self, out, in_, rnd=False):
        """PSUM->SBUF copy, round-robin between DVE and ACT to balance
        engine load. rnd rounds to f32r."""
        if rnd and self.use_f32r:
            out = out.bitcast(F32R)
        Builder._rr += 1
        if Builder._rr % 2 == 0:
            self.nc.vector.tensor_copy(out=out, in_=in_)
        else:
            self.nc.scalar.copy(out=out, in_=in_)

    # ---------- generic helpers ----------

    def transpose_tiles(self, src, dst, rnd=False):
        """dst[i][:, j*128:(j+1)*128] = src[j][:, i*128:(i+1)*128].T
        rnd=True rounds the copy-out to float32r (for matmul operands)."""
        nc = self.nc
        rnd = rnd and self.use_f32r
        for i in range(len(dst)):
            for j in range(len(src)):
                pt = self.pt.tile([P, P], F32, tag="pt", name="pt")
                nc.tensor.transpose(pt, src[j][:, i * P:(i + 1) * P], self.ident)
                self.copy_any(dst[i][:, j * P:(j + 1) * P], pt, rnd=rnd)

    def l2norm_rows(self, src, dst):
        """Row-l2-normalize node-major tiles src -> dst ([128, C] each)."""
        nc = self.nc
        for s, d in zip(src, dst):
            ss = self.col.tile([P, 1], F32, tag="ss", name="ss")
            sq = self.work.tile([P, C], F32, tag="scratch", name="sq")
            nc.scalar.activation(out=sq, in_=s, func=AF.Square, accum_out=ss)
            rn = self.col.tile([P, 1], F32, tag="rn", name="rn")
            nc.scalar.sqrt(out=rn, in_=ss)
            nc.vector.tensor_scalar_max(out=rn, in0=rn, scalar1=EPS)
            nc.vector.reciprocal(out=rn, in_=rn)
            nc.vector.tensor_scalar_mul(out=d, in0=s, scalar1=rn)

    def layernorm_lrelu(self, in_ap, out_ap, g_row, b_row, offload=False):
        """LN over free dim (512) then leaky-relu 0.01. in_ap PSUM/SBUF.
        offload=True routes the SBUF-only scale/shift ops to GpSimd to
        relieve the vector engine (GpSimd cannot touch PSUM)."""
        nc = self.nc
        ew = nc.gpsimd if offload else nc.vector
        stats = self.col.tile([P, 6], F32, tag="bnst", name="bnst")
        nc.vector.bn_stats(out=stats, in_=in_ap)
        mv = self.col.tile([P, 2], F32, tag="bnmv", name="bnmv")
        nc.vector.bn_aggr(out=mv, in_=stats)
        rstd = self.col.tile([P, 1], F32, tag="rstd", name="rstd")
        nc.scalar.activation(out=rstd, in_=mv[:, 1:2], func=AF.Sqrt,
                             bias=self.eps_ln)
        nc.vector.reciprocal(out=rstd, in_=rstd)
        nmr = self.col.tile([P, 1], F32, tag="nmr", name="nmr")
        nc.vector.tensor_tensor(out=nmr, in0=mv[:, 0:1], in1=rstd,
                                op=ALU.mult)
        nc.scalar.mul(out=nmr, in_=nmr, mul=-1.0)
        # z = (in - m) * rstd  ==  in * rstd + (-m * rstd)   (ACT pass)
        z = self.work.tile([P, C], F32, tag="scratch", name="lnz")
        nc.scalar.activation(out=z, in_=in_ap, func=AF.Identity,
                             scale=rstd, bias=nmr)
        ew.tensor_tensor(out=z, in0=z, in1=g_row, op=ALU.mult)
        if self.use_lrelu_act and not offload:
            zb2 = self.work.tile([P, C], F32, tag="scratch", name="lnzb")
            ew.tensor_tensor(out=zb2, in0=z, in1=b_row, op=ALU.add)
            nc.scalar.activation(out=out_ap, in_=zb2, func=AF.Lrelu,
                                 alpha=0.01)
        else:
            ew.tensor_tensor(out=z, in0=z, in1=b_row, op=ALU.add)
            # lrelu(x) = max(x, 0.01x); imm tensor_scalar can't encode on
            # Pool, so the 0.01 mul stays on DVE
            z2 = self.work.tile([P, C], F32, tag="scratch", name="lnz2")
            nc.vector.tensor_scalar_mul(out=z2, in0=z, scalar1=0.01)
            nc.vector.tensor_tensor(out=out_ap, in0=z, in1=z2, op=ALU.max)

    def matmul_nt(self, lhsT_slices, rhs_tiles, out_cb, nfree=C):
        """out = sum_k lhsT_slices[k].T @ rhs_tiles[k] -> out_cb(psum)."""
        nc = self.nc
        ps = self.pm.tile([P, nfree], F32, tag="pm", name="pm")
        nk = len(lhsT_slices)
        for k in range(nk):
            self.mm(ps, lhsT_slices[k], rhs_tiles[k],
                    start=(k == 0), stop=(k == nk - 1))
        out_cb(ps)

    def xw_plus(self, xT, wT, out_raw=None, out_biased=None, b_row=None):
        """z = x @ W^T (+ b) from transposed input tiles xT."""
        nc = self.nc
        for m in range(NT):
            def cb(ps, m=m):
                if out_raw is not None:
                    self.copy_any(out_raw[m], ps, rnd=True)
                if out_biased is not None:
                    nc.vector.tensor_tensor(out=out_biased[m], in0=ps,
                                            in1=b_row, op=ALU.add)
            self.matmul_nt(
                [xT[k][:, m * P:(m + 1) * P] for k in range(CT)], wT, cb)


def _load_w_tiles(nc, B, pool, w_dram):
    tiles = []
    for k in range(CT):
        stg = pool.tile([P, C], F32, tag="wstage", name="wstage", bufs=2)
        nc.sync.dma_start(out=stg, in_=w_dram[k * P:(k + 1) * P, :])
        t = pool.tile([P, C], F32, tag="wmat", name="wmat")
        B.rcopy(t, stg)
        tiles.append(t)
    return tiles


def build_nc(phases=("dp", "tc", "bi", "fa")):
    phases = set(phases)
    nc = bass.Bass()

    # ----- DRAM I/O -----
    x_d = nc.dram_tensor("x", [N, C], F32, kind="ExternalInput")
    g_d = nc.dram_tensor("g", [GN, C], F32, kind="ExternalInput")
    pos_d = nc.dram_tensor("pos", [N, POSD], F32, kind="ExternalInput")
    wv_d = nc.dram_tensor("wvec", [5], F32, kind="ExternalInput")

    graph_ops = ["dp", "fa1", "fa2", "fa3"]
    wT_d, b_d, g_ln_d, bln_d = {}, {}, {}, {}
    for o in graph_ops:
        wT_d[o] = nc.dram_tensor(f"{o}_wT", [C, C], F32, kind="ExternalInput")
        b_d[o] = nc.dram_tensor(f"{o}_b", [C], F32, kind="ExternalInput")
        g_ln_d[o] = nc.dram_tensor(f"{o}_g", [C], F32, kind="ExternalInput")
        bln_d[o] = nc.dram_tensor(f"{o}_bln", [C], F32, kind="ExternalInput")
    bi_w1T_d = nc.dram_tensor("bi_w1T", [C, C], F32, kind="ExternalInput")
    bi_w2T_d = nc.dram_tensor("bi_w2T", [M, C], F32, kind="ExternalInput")
    bi_b_d = nc.dram_tensor("bi_b", [C], F32, kind="ExternalInput")
    bi_g_d = nc.dram_tensor("bi_g", [C], F32, kind="ExternalInput")
    bi_bln_d = nc.dram_tensor("bi_bln", [C], F32, kind="ExternalInput")
    tc_cw_d = nc.dram_tensor("tc_cw", [KW, C], F32, kind="ExternalInput")
    tc_cb_d = nc.dram_tensor("tc_cb", [C], F32, kind="ExternalInput")
    tc_g_d = nc.dram_tensor("tc_g", [C], F32, kind="ExternalInput")
    tc_bln_d = nc.dram_tensor("tc_bln", [C], F32, kind="ExternalInput")
    na_wA_d, na_wp_d, na_b_d = {}, {}, {}
    for i in (1, 2, 3):
        na_wA_d[i] = nc.dram_tensor(f"na{i}_wA", [N, 1], F32, kind="ExternalInput")
        na_wp_d[i] = nc.dram_tensor(f"na{i}_wp", [POSD], F32, kind="ExternalInput")
        na_b_d[i] = nc.dram_tensor(f"na{i}_b", [1], F32, kind="ExternalInput")

    y_d = nc.dram_tensor("y", [N, C], F32, kind="ExternalOutput")

    with tile.TileContext(nc) as tcx:
        B = Builder(nc)
        with (
            tcx.tile_pool(name="pers", bufs=1) as pers,
            tcx.tile_pool(name="wpool", bufs=8) as wpool,
            tcx.tile_pool(name="work", bufs=8) as work,
            tcx.tile_pool(name="col", bufs=8) as col,
            tcx.tile_pool(name="pm", bufs=3, space="PSUM") as pm,
            tcx.tile_pool(name="pt", bufs=3, space="PSUM") as pt,
        ):
            B.work, B.col, B.pm, B.pt = work, col, pm, pt

            ident = pers.tile([P, P], F32, tag="ident", name="ident")
            make_identity(nc, ident)
            B.ident = ident
            identb = pers.tile([P, P], mybir.dt.bfloat16, tag="identb",
                               name="identb")
            make_identity(nc, identb)
            eps_ln = pers.tile([P, 1], F32, tag="epsln", name="epsln")
            nc.vector.memset(eps_ln, 1e-5)
            B.eps_ln = eps_ln

            X = [pers.tile([P, C], F32, tag=f"X{m}", name=f"X{m}")
                 for m in range(NT)]
            for m in range(NT):
                nc.sync.dma_start(out=X[m], in_=x_d[m * P:(m + 1) * P, :])

            wvrow = _bcast_row(nc, pers, wv_d[:], 5, tag="wvrow")
            wv = [wvrow[:, i:i + 1] for i in range(5)]

            pos_t = [pers.tile([P, POSD], F32, tag=f"pos{m}", name=f"pos{m}")
                     for m in range(NT)]
            for m in range(NT):
                nc.sync.dma_start(out=pos_t[m], in_=pos_d[m * P:(m + 1) * P, :])

            # branch outputs (consumed by fa1/2/3 later)
            dpout = [pers.tile([P, C], F32, tag=f"dpo{m}", name=f"dpo{m}")
                     for m in range(NT)]
            tcacc = [pers.tile([P, C], F32, tag=f"tca{m}", name=f"tca{m}")
                     for m in range(NT)]
            biout = [pers.tile([P, C], F32, tag=f"bio{m}", name=f"bio{m}")
                     for m in range(NT)]
            out_acc = [pers.tile([P, C], F32, tag=f"oacc{m}", name=f"oacc{m}")
                       for m in range(NT)]
            for m in range(NT):
                nc.vector.tensor_scalar_mul(out=out_acc[m], in0=X[m],
                                            scalar1=wv[1])

            with tcx.tile_pool(name="early", bufs=1) as early:
                # nf, nfT, S shared by dp / tc / bi
                nf = [early.tile([P, C], F32, tag=f"nf{m}", name=f"nf{m}")
                      for m in range(NT)]
                B.l2norm_rows(X, nf)
                nfT = [early.tile([P, N], F32, tag=f"nfT{k}", name=f"nfT{k}")
                       for k in range(CT)]
                B.transpose_tiles(nf, nfT, rnd=True)
                S_sb = [early.tile([P, N], F32, tag=f"S{m}", name=f"S{m}")
                        for m in range(NT)]
                for m in range(NT):
                    B.matmul_nt(
                        [nfT[k][:, m * P:(m + 1) * P] for k in range(CT)],
                        nfT,
                        lambda ps, m=m: B.copy_any(S_sb[m], ps),
                        nfree=N)

                # ===================== diff_prop =====================
                if "dp" not in phases:
                    for m in range(NT):
                        nc.vector.memset(dpout[m], 0.0)
                if "dp" in phases:
                  with tcx.tile_pool(name="dpp", bufs=1) as dpp:
                    xT = [dpp.tile([P, N], F32, tag=f"xT{k}", name=f"xT{k}")
                          for k in range(CT)]
                    B.transpose_tiles(X, xT, rnd=True)
                    zraw = [dpp.tile([P, C], F32, tag=f"zr{m}", name=f"zr{m}")
                            for m in range(NT)]
                    zb = [dpp.tile([P, C], F32, tag=f"zb{m}", name=f"zb{m}")
                          for m in range(NT)]
                    dpb = _bcast_row(nc, dpp, b_d["dp"][:], C, tag="dpb")
                    dpg = _bcast_row(nc, dpp, g_ln_d["dp"][:], C, tag="dpg")
                    dpbl = _bcast_row(nc, dpp, bln_d["dp"][:], C, tag="dpbl")
                    wT = _load_w_tiles(nc, B, wpool, wT_d["dp"])
                    B.xw_plus(xT, wT, out_raw=zraw, out_biased=zb, b_row=dpb)

                    E = [dpp.tile([P, N], F32, tag=f"E{m}", name=f"E{m}")
                         for m in range(NT)]
                    s_cols, recn_cols = [], []
                    for m in range(NT):
                        fs = col.tile([P, 1], F32, tag="dpfs", name="dpfs")
                        # mask diagonal to -inf before exp -> exp gives 0
                        Sd = work.tile([P, N], F32, tag="scratch", name="Sd")
                        nc.gpsimd.affine_select(
                            out=Sd, in_=S_sb[m], compare_op=ALU.not_equal,
                            fill=-1e30, base=m * P, pattern=[[-1, N]],
                            channel_multiplier=1)
                        eout = (E[m].bitcast(F32R) if B.use_f32r else E[m])
                        nc.scalar.activation(out=eout, in_=Sd,
                                             func=AF.Exp, scale=5.0,
                                             accum_out=fs)
                        rec = col.tile([P, 1], F32, tag="dprec", name="dprec")
                        nc.vector.tensor_scalar_max(out=rec, in0=fs, scalar1=EPS)
                        nc.vector.reciprocal(out=rec, in_=rec)
                        s_col = col.tile([P, 1], F32, tag="dps", name="dps")
                        nc.vector.tensor_tensor(out=s_col, in0=fs, in1=rec,
                                                op=ALU.mult)
                        recn = col.tile([P, 1], F32, tag="dprecn", name="dprecn")
                        nc.scalar.mul(out=recn, in_=rec, mul=-1.0)
                        s_cols.append(s_col)
                        recn_cols.append(recn)

                    for m in range(NT):
                        szb = work.tile([P, C], F32, tag="scratch", name="szb")
                        nc.vector.tensor_scalar_mul(out=szb, in0=zb[m],
                                                    scalar1=s_cols[m])

                        def cb(ps, m=m, szb=szb):
                            yt = work.tile([P, C], F32, tag="scratch",
                                           name="dpy")
                            nc.vector.scalar_tensor_tensor(
                                out=yt, in0=ps, scalar=recn_cols[m], in1=szb,
                                op0=ALU.mult, op1=ALU.add)
                            B.layernorm_lrelu(yt, dpout[m], dpg, dpbl)
                        B.matmul_nt(
                            [E[k][:, m * P:(m + 1) * P] for k in range(NT)],
                            zraw, cb)

                # ===================== temp_conv =====================
                if "tc" not in phases:
                    for m in range(NT):
                        nc.vector.memset(tcacc[m], 0.0)
                if "tc" in phases:
                  with tcx.tile_pool(name="tcp", bufs=1) as tcp:
                    BF16 = mybir.dt.bfloat16
                    # one-hot(argmax per frame) in bf16 (exact 0/1)
                    OH = [tcp.tile([P, N], BF16, tag=f"OH{m}", name=f"OH{m}")
                          for m in range(NT)]
                    for m in range(NT):
                        gmax = col.tile([P, T], F32, tag="gmax", name="gmax")
                        s3 = S_sb[m].rearrange("p (t n) -> p t n", t=T)
                        nc.vector.reduce_max(out=gmax, in_=s3, axis=AX.X)
                        g3 = gmax.rearrange("p (t o) -> p t o", o=1)
                        nc.vector.tensor_tensor(
                            out=OH[m].rearrange("p (t n) -> p t n", t=T),
                            in0=s3, in1=g3.broadcast_to((P, T, NPF)),
                            op=ALU.is_equal)
                    OHT = [tcp.tile([P, N], BF16, tag=f"OHT{m}",
                                    name=f"OHT{m}") for m in range(NT)]
                    for i in range(NT):
                        for j in range(NT):
                            ptb = pt.tile([P, P], BF16, tag="pt", name="ptb")
                            nc.tensor.transpose(
                                ptb, OH[j][:, i * P:(i + 1) * P], identb)
                            nc.vector.tensor_copy(
                                out=OHT[i][:, j * P:(j + 1) * P], in_=ptb)

                    cw_rows = [_bcast_row(nc, tcp, tc_cw_d[k, :], C,
                                          tag=f"cw{k}") for k in range(KW)]
                    tccb = _bcast_row(nc, tcp, tc_cb_d[:], C, tag="tccb")
                    tcg = _bcast_row(nc, tcp, tc_g_d[:], C, tag="tcg")
                    tcbl = _bcast_row(nc, tcp, tc_bln_d[:], C, tag="tcbl")

                    # Shift-packed operands: row r=(k,j) of column t holds
                    # frame (t+k): OH4/W4 for taps 0-3 (K=128), OH3/W3 for
                    # taps 4-6 (K=96). Repacking via SBUF->SBUF DMA.
                    OH4 = tcp.tile([P, T, N], BF16, tag="OH4", name="OH4")
                    OH3 = tcp.tile([96, T, N], BF16, tag="OH3", name="OH3")
                    W4 = tcp.tile([P, T, C], BF16, tag="W4", name="W4")
                    W3 = tcp.tile([96, T, C], BF16, tag="W3", name="W3")
                    for k in range(KW):
                        dstOH = OH4 if k < 4 else OH3
                        ro = 32 * k if k < 4 else 32 * (k - 4)
                        for t in range(T - k):
                            tp = t + k
                            tl, off = tp // 4, (tp % 4) * NPF
                            nc.sync.dma_start(
                                out=dstOH[ro:ro + NPF, t, :],
                                in_=OHT[tl][off:off + NPF, :])
                    with tcx.tile_pool(name="xwp", bufs=2) as xwp:
                        for k in range(KW):
                            dstW = W4 if k < 4 else W3
                            ro = 32 * k if k < 4 else 32 * (k - 4)
                            Xwk = [xwp.tile([P, C], BF16, tag="xwk",
                                            name="xwk") for _ in range(NT)]
                            for mm in range(NT):
                                nc.vector.tensor_tensor(
                                    out=Xwk[mm], in0=X[mm], in1=cw_rows[k],
                                    op=ALU.mult)
                            for t in range(T - k):
                                tp = t + k
                                tl, off = tp // 4, (tp % 4) * NPF
                                nc.sync.dma_start(
                                    out=dstW[ro:ro + NPF, t, :],
                                    in_=Xwk[tl][off:off + NPF, :])

                    for m in range(NT):
                        for to in range(TO):
                            ps = pm.tile([P, C], F32, tag="pm", name="pm")
                            nc.tensor.matmul(
                                ps, OH4[:, to, m * P:(m + 1) * P],
                                W4[:, to, :], start=True, stop=False)
                            nc.tensor.matmul(
                                ps, OH3[:, to, m * P:(m + 1) * P],
                                W3[:, to, :], start=False, stop=True)
                            yb = work.tile([P, C], F32, tag="scratch",
                                           name="tcyb")
                            nc.vector.tensor_tensor(out=yb, in0=ps, in1=tccb,
                                                    op=ALU.add)
                            zt = work.tile([P, C], F32, tag="scratch",
                                           name="tczt")
                            B.layernorm_lrelu(yb, zt, tcg, tcbl,
                                              offload=True)
                            if to == 0:
                                nc.scalar.copy(out=tcacc[m], in_=zt)
                            else:
                                nc.vector.tensor_tensor(out=tcacc[m],
                                                        in0=tcacc[m], in1=zt,
                                                        op=ALU.add)
                        nc.scalar.mul(out=tcacc[m], in_=tcacc[m], mul=1.0 / TO)

                # ===================== back_incor ====================
                if "bi" not in phases:
                    for m in range(NT):
                        nc.vector.memset(biout[m], 0.0)
                if "bi" in phases:
                  with tcx.tile_pool(name="bip", bufs=1) as bip:
                    Gt = [bip.tile([P, C], F32, tag=f"G{i}", name=f"G{i}")
                          for i in range(GT_TILES)]
                    nc.gpsimd.memset(Gt[GT_TILES - 1], 0.0)
                    for i in range(GT_TILES):
                        r0, r1 = i * P, min((i + 1) * P, GN)
                        nc.sync.dma_start(out=Gt[i][:r1 - r0, :],
                                          in_=g_d[r0:r1, :])
                    nfg = [bip.tile([P, C], F32, tag=f"nfg{i}", name=f"nfg{i}")
                           for i in range(GT_TILES)]
                    B.l2norm_rows(Gt, nfg)
                    GNP = GT_TILES * P
                    nfgT = [bip.tile([P, GNP], F32, tag=f"nfgT{k}",
                                     name=f"nfgT{k}") for k in range(CT)]
                    B.transpose_tiles(nfg, nfgT, rnd=True)

                    u_stack = [bip.tile([P, C], F32, tag=f"bu{m}",
                                        name=f"bu{m}") for m in range(NT)]
                    R = [bip.tile([P, M], F32, tag=f"bR{m}", name=f"bR{m}")
                         for m in range(NT)]
                    with tcx.tile_pool(name="gpp", bufs=3) as gpp:
                        for t in range(T):
                            mtile, moff = t // 4, (t % 4) * NPF
                            gpad = gpp.tile([M, C], F32, tag="gpad",
                                            name="gpad")
                            nc.sync.dma_start(out=gpad,
                                              in_=g_d[t * M:(t + 1) * M, :])
                            # Araw_t [32, 49] (raw, for concat path)
                            psA = pt.tile([P, M], F32, tag="pt", name="psA")
                            for k in range(CT):
                                B.mm(psA[:NPF, :],
                                     nfT[k][:, t * NPF:(t + 1) * NPF],
                                     nfgT[k][:, t * M:(t + 1) * M],
                                     start=(k == 0), stop=(k == CT - 1))
                            fs = col.tile([P, 1], F32, tag="bifs", name="bifs")
                            scr = work.tile([P, M], F32, tag="biscr",
                                            name="biscr")
                            nc.scalar.activation(out=scr[:NPF, :],
                                                 in_=psA[:NPF, :],
                                                 func=AF.Exp, scale=5.0,
                                                 accum_out=fs[:NPF, :])
                            nc.scalar.copy(out=R[mtile][moff:moff + NPF, :],
                                           in_=psA[:NPF, :])
                            rs = col.tile([P, 1], F32, tag="birs", name="birs")
                            nc.vector.reciprocal(out=rs[:NPF, :],
                                                 in_=fs[:NPF, :])
                            # ArawT_t [49, 32] -> FT = exp(5 ArawT)
                            psAT = pt.tile([P, NPF], F32, tag="pt", name="psAT")
                            for k in range(CT):
                                B.mm(psAT[:M, :],
                                     nfgT[k][:, t * M:(t + 1) * M],
                                     nfT[k][:, t * NPF:(t + 1) * NPF],
                                     start=(k == 0), stop=(k == CT - 1))
                            FT = work.tile([P, NPF], F32, tag="biFT",
                                           name="biFT")
                            fto = (FT[:M, :].bitcast(F32R) if B.use_f32r
                                   else FT[:M, :])
                            nc.scalar.activation(out=fto, in_=psAT[:M, :],
                                                 func=AF.Exp, scale=5.0)
                            gpr = gpp.tile([M, C], F32, tag="gpr",
                                           name="gpr")
                            B.rcopy(gpr, gpad)
                            # u_t = rs * (FT.T @ gf_t)
                            psu = pm.tile([P, C], F32, tag="pm", name="psu")
                            B.mm(psu[:NPF, :], FT[:M, :], gpr,
                                 start=True, stop=True)
                            nc.vector.tensor_scalar_mul(
                                out=u_stack[mtile][moff:moff + NPF, :],
                                in0=psu[:NPF, :], scalar1=rs[:NPF, :])

                    uT = [bip.tile([P, N], F32, tag=f"buT{k}", name=f"buT{k}")
                          for k in range(CT)]
                    B.transpose_tiles(u_stack, uT, rnd=True)
                    RT = bip.tile([M, N], F32, tag="bRT", name="bRT")
                    for m in range(NT):
                        ptr = pt.tile([P, P], F32, tag="pt", name="ptr")
                        nc.tensor.transpose(ptr[:M, :], R[m], ident)
                        B.rcopy(RT[:, m * P:(m + 1) * P], ptr[:M, :])

                    w1T = _load_w_tiles(nc, B, wpool, bi_w1T_d)
                    w2s = wpool.tile([M, C], F32, tag="w2s", name="w2s",
                                     bufs=1)
                    nc.sync.dma_start(out=w2s, in_=bi_w2T_d[:, :])
                    w2T = wpool.tile([M, C], F32, tag="w2", name="w2", bufs=1)
                    B.rcopy(w2T, w2s)
                    bib = _bcast_row(nc, bip, bi_b_d[:], C, tag="bib")
                    big_ = _bcast_row(nc, bip, bi_g_d[:], C, tag="big")
                    bibl = _bcast_row(nc, bip, bi_bln_d[:], C, tag="bibl")
                    for m in range(NT):
                        ps = pm.tile([P, C], F32, tag="pm", name="pm")
                        for k in range(CT):
                            B.mm(ps, uT[k][:, m * P:(m + 1) * P],
                                 w1T[k], start=(k == 0), stop=False)
                        B.mm(ps, RT[:, m * P:(m + 1) * P], w2T,
                             start=False, stop=True)
                        yb = work.tile([P, C], F32, tag="scratch", name="biyb")
                        nc.vector.tensor_tensor(out=yb, in0=ps, in1=bib,
                                                op=ALU.add)
                        B.layernorm_lrelu(yb, biout[m], big_, bibl)

            # ============== feat_aggr + node_att branches ==============
            for idx, xin in (((1, dpout), (2, tcacc), (3, biout))
                             if "fa" in phases else ()):
                fa = f"fa{idx}"
                with tcx.tile_pool(name=f"br{idx}", bufs=1) as brp:
                    faout = _feat_aggr(nc, B, brp, wpool, work, col, xin,
                                       wT_d[fa], b_d[fa], g_ln_d[fa],
                                       bln_d[fa])
                    _node_att(nc, B, brp, work, col, faout, pos_t,
                              na_wA_d[idx], na_wp_d[idx], na_b_d[idx],
                              wv[idx + 1], out_acc)

            for m in range(NT):
                nc.sync.dma_start(out=y_d[m * P:(m + 1) * P, :],
                                  in_=out_acc[m])

    split_multiwaits(nc)
    return nc


def _feat_aggr(nc, B, brp, wpool, work, col, xin, wT_dram, b_dram,
               g_dram, bln_dram):
    """y = lrelu(LN(softmax(5 nf nf^T) @ (x W^T + b)))"""
    xinT = [brp.tile([P, N], F32, tag=f"fxT{k}", name=f"fxT{k}")
            for k in range(CT)]
    B.transpose_tiles(xin, xinT, rnd=True)
    nfp = [brp.tile([P, C], F32, tag=f"fnf{m}", name=f"fnf{m}")
           for m in range(NT)]
    B.l2norm_rows(xin, nfp)
    nfpT = [brp.tile([P, N], F32, tag=f"fnfT{k}", name=f"fnfT{k}")
            for k in range(CT)]
    B.transpose_tiles(nfp, nfpT, rnd=True)

    wT = _load_w_tiles(nc, B, wpool, wT_dram)
    fab = _bcast_row(nc, brp, b_dram[:], C, tag="fab")
    fag = _bcast_row(nc, brp, g_dram[:], C, tag="fag")
    fabl = _bcast_row(nc, brp, bln_dram[:], C, tag="fabl")
    # bias moves out of the matmul: softmax rows sum to 1, so
    # A@(z + 1 b^T) = A@z + b  -> apply b after the aggregation.
    z = [brp.tile([P, C], F32, tag=f"fz{m}", name=f"fz{m}")
         for m in range(NT)]
    B.xw_plus(xinT, wT, out_raw=z)

    F = [brp.tile([P, N], F32, tag=f"fF{m}", name=f"fF{m}") for m in range(NT)]
    rs_cols = []
    for m in range(NT):
        fs = col.tile([P, 1], F32, tag="fafs", name="fafs")

        def cb(ps, m=m, fs=fs):
            fo = F[m].bitcast(F32R) if B.use_f32r else F[m]
            nc.scalar.activation(out=fo, in_=ps, func=AF.Exp, scale=5.0,
                                 accum_out=fs)
        B.matmul_nt([nfpT[k][:, m * P:(m + 1) * P] for k in range(CT)],
                    nfpT, cb, nfree=N)
        rs = col.tile([P, 1], F32, tag="fars", name="fars")
        nc.vector.reciprocal(out=rs, in_=fs)
        rs_cols.append(rs)

    out = []
    for m in range(NT):
        o = brp.tile([P, C], F32, tag=f"fo{m}", name=f"fo{m}")

        def cb(ps, m=m, o=o):
            yt = work.tile([P, C], F32, tag="scratch", name="fayt")
            nc.vector.scalar_tensor_tensor(out=yt, in0=ps,
                                           scalar=rs_cols[m], in1=fab,
                                           op0=ALU.mult, op1=ALU.add)
            B.layernorm_lrelu(yt, o, fag, fabl)
        B.matmul_nt([F[k][:, m * P:(m + 1) * P] for k in range(NT)], z, cb)
        out.append(o)
    return out


def _node_att(nc, B, brp, work, col, xin, pos_t, wA_dram, wp_dram, b_dram,
              wv_col, out_acc):
    """out_acc += wv * (xin * sigmoid(nf@(nf^T@wA) + pos@wp + b))
    A@wA == nf @ (nf^T @ wA): two PE matvecs, A never materialized."""
    nfp = [brp.tile([P, C], F32, tag=f"nnf{m}", name=f"nnf{m}")
           for m in range(NT)]
    B.l2norm_rows(xin, nfp)
    nfpT = [brp.tile([P, N], F32, tag=f"nnfT{k}", name=f"nnfT{k}")
            for k in range(CT)]
    B.transpose_tiles(nfp, nfpT)

    wA = []
    for i in range(NT):
        t = brp.tile([P, 1], F32, tag=f"nwA{i}", name=f"nwA{i}")
        nc.sync.dma_start(out=t, in_=wA_dram[i * P:(i + 1) * P, :])
        wA.append(t)
    wp_row = _bcast_row(nc, brp, wp_dram[:], POSD, tag="nwp")
    b_row = _bcast_row(nc, brp, b_dram[:], 1, tag="nb")

    # v = nf^T @ wA (channel-major column, CT pieces [128, 1])
    v = []
    for c in range(CT):
        pv = B.pt.tile([P, 1], F32, tag="pt", name="pv")
        for m in range(NT):
            B.mm(pv, nfp[m][:, c * P:(c + 1) * P], wA[m],
                 start=(m == 0), stop=(m == NT - 1))
        vc = col.tile([P, 1], F32, tag="nav", name="nav")
        nc.scalar.copy(out=vc, in_=pv)
        v.append(vc)

    for m in range(NT):
        pa = B.pt.tile([P, 1], F32, tag="pt", name="pa")
        for k in range(CT):
            B.mm(pa, nfpT[k][:, m * P:(m + 1) * P], v[k],
                 start=(k == 0), stop=(k == CT - 1))
        sp = work.tile([P, POSD], F32, tag="naps", name="naps")
        nc.vector.tensor_tensor(out=sp, in0=pos_t[m], in1=wp_row, op=ALU.mult)
        pw = col.tile([P, 1], F32, tag="napw", name="napw")
        nc.vector.reduce_sum(out=pw, in_=sp, axis=AX.X)
        att = col.tile([P, 1], F32, tag="naatt", name="naatt")
        nc.vector.tensor_tensor(out=att, in0=pa, in1=pw, op=ALU.add)
        nc.scalar.activation(out=pw, in_=att, func=AF.Sigmoid,
                             bias=b_row, scale=1.0)
        contrib = work.tile([P, C], F32, tag="scratch", name="nact")
        nc.vector.tensor_scalar(out=contrib, in0=xin[m], scalar1=pw,
                                scalar2=wv_col, op0=ALU.mult, op1=ALU.mult)
        nc.gpsimd.tensor_tensor(out=out_acc[m], in0=out_acc[m], in1=contrib,
                                op=ALU.add)


# =====================================================================
# host wrapper
# =====================================================================

_NC_CACHE = {}


def _get_nc():
    if "nc" not in _NC_CACHE:
        _NC_CACHE["nc"] = build_nc()
    return _NC_CACHE["nc"]


def make_in_maps(local_feat, global_feat, pos, weights, params):
    def ap(v):
        return np.ascontiguousarray(np.asarray(v, dtype=np.float32))

    shared = {"wvec": ap(weights)}
    for o in ("dp", "fa1", "fa2", "fa3"):
        p = params[o]
        assert np.allclose(np.diagonal(ap(p["adj_w"])), 1.0), "adj diag != 1"
        shared[f"{o}_wT"] = ap(np.asarray(p["aff_w"]).T)
        shared[f"{o}_b"] = ap(p["aff_b"])
        shared[f"{o}_g"] = ap(p["ln_g"])
        shared[f"{o}_bln"] = ap(p["ln_b"])
    pbi = params["bi"]
    assert np.allclose(np.diagonal(ap(pbi["adj_w"])), 1.0)
    wbi = ap(pbi["aff_w"])          # [512, 561]
    shared["bi_w1T"] = ap(wbi[:, :C].T)
    shared["bi_w2T"] = ap(wbi[:, C:].T)
    shared["bi_b"] = ap(pbi["aff_b"])
    shared["bi_g"] = ap(pbi["ln_g"])
    shared["bi_bln"] = ap(pbi["ln_b"])
    ptc = params["tc"]
    assert np.allclose(np.diagonal(ap(ptc["adj_w"])), 1.0)
    shared["tc_cw"] = ap(np.asarray(ptc["conv_w"])[:, 0, :].T)   # [7, 512]
    shared["tc_cb"] = ap(ptc["conv_b"])
    shared["tc_g"] = ap(ptc["ln_g"])
    shared["tc_bln"] = ap(ptc["ln_b"])
    for i, na in ((1, "na1"), (2, "na2"), (3, "na3")):
        att_w = ap(params[na]["att_w"])[0]      # [521]
        shared[f"na{i}_wA"] = ap(att_w[:N][:, None])
        shared[f"na{i}_wp"] = ap(att_w[N:])
        shared[f"na{i}_b"] = ap(params[na]["att_b"])

    in_maps = []
    for b in range(local_feat.shape[0]):
        m = dict(shared)
        m["x"] = ap(local_feat[b].reshape(N, C))
        m["g"] = ap(global_feat[b].reshape(GN, C))
        m["pos"] = ap(pos[b].reshape(N, POSD))
        in_maps.append(m)
    return in_maps


def kernel(local_feat, global_feat, pos, weights, params):
    from concourse.bass_utils import run_bass_kernel_spmd

    local_feat = np.asarray(local_feat, dtype=np.float32)
    global_feat = np.asarray(global_feat, dtype=np.float32)
    pos = np.asarray(pos, dtype=np.float32)
    weights = np.asarray(weights, dtype=np.float32)

    in_maps = make_in_maps(local_feat, global_feat, pos, weights, params)
    nc = _get_nc()
    res = run_bass_kernel_spmd(nc, in_maps, core_ids=list(range(8)))
    out = np.stack([res.results[b]["y"].reshape(T, NPF, C)
                    for b in range(8)])
    return out.astype(np.float32)


# revision 28
# speedup vs baseline: 1.1497x; 1.1497x over previous
"""Trainium2 Bass kernel for nn_MixedOp (gnn_message_passing).

Strategy: data-parallel over batch (B=8 -> 1 sample/core). Everything on
device except zero-FLOP host marshaling (reshapes / weight transposes /
diagonal extraction; adj_w is the identity so diag(adj_w) == 1).

Math restructuring (exact up to FP reassociation):
- softmax(5A)@Z = (1/rowsum(exp(5S))) * (exp(5S) @ Z) with S = nf@nf^T
  symmetric -> exp(5S) symmetric -> its tiles usable as matmul lhsT
  directly (no attention-matrix transpose).
- diff_prop: y = s*(xW+b) - rec*(E@(xW)) with E = exp(5S) diag-zeroed,
  rec = 1/max(rowsum E, eps), s = rowsum(E)*rec.
- node_att: A@wA = nf @ (nf^T @ wA): two matvecs, A never materialized.
- temp_conv: one-hot(argmax) per 32-node frame; gather + depthwise conv
  folded into PSUM-accumulated matmuls: y_to = sum_k OH_{to+k}^T-slice @
  (X * w_k)_{to+k}-rows.
"""

import sys

for _p in ("/opt/trn_rl_repo",):
    if _p not in sys.path:
        sys.path.insert(0, _p)

import numpy as np

import concourse.bass as bass
import concourse.tile as tile
from concourse import mybir
from concourse.masks import make_identity

F32 = mybir.dt.float32
AF = mybir.ActivationFunctionType
ALU = mybir.AluOpType
AX = mybir.AxisListType

P = 128
N = 512          # nodes per sample (T*NODES)
C = 512          # channels
NT = N // P      # node tiles
CT = C // P      # channel tiles
T = 16           # frames
NPF = 32         # nodes per frame
M = 49           # global nodes per frame
GN = T * M       # 784 global rows
GT_TILES = (GN + P - 1) // P  # 7
KW = 7           # conv taps
TO = T - KW + 1  # 10 conv outputs
POSD = 9
EPS = 1e-12


def split_multiwaits(nc):
    """This walrus build accepts at most ONE sync-wait per instruction;
    Tile emits several. Hoist all but the last onto same-engine NOPs
    placed immediately before the instruction (same blocking semantics;
    each engine's program order is a topological order of the dep graph,
    so the hoisted waits cannot deadlock)."""
    n = 0
    for fn in nc.m.functions:
        for bb in fn.blocks:
            new_list = []
            for inst in bb.instructions:
                si = inst.sync_info
                if si is not None and si.on_wait and len(si.on_wait) > 1:
                    waits = list(si.on_wait)
                    for w in waits[:-1]:
                        nop = mybir.InstNoOp(
                            name=f"I-splitw-{n}",
                            engine=inst.engine,
                            ins=[],
                            outs=[],
                            sync_info=mybir.SyncInfo(on_wait=[w], on_update=[]),
                        )
                        n += 1
                        nc.register_instruction(nop)
                        new_list.append(nop)
                    inst.sync_info = mybir.SyncInfo(
                        on_wait=[waits[-1]], on_update=si.on_update
                    )
                new_list.append(inst)
            bb.instructions[:] = new_list
    return n


def _bcast_row(nc, pool, dram_ap, length, tag="brow"):
    """DMA a [length] DRAM vector to SBUF broadcast across 128 partitions."""
    t = pool.tile([P, length], F32, tag=tag, name=tag)
    src = bass.AP(tensor=dram_ap.tensor, offset=dram_ap.offset,
                  ap=[[0, P], [1, length]])
    nc.sync.dma_start(out=t, in_=src)
    return t


F32R = mybir.dt.float32r


class Builder:
    def __init__(self, nc):
        self.nc = nc
        import os
        self.use_f32r = os.environ.get("KF32R", "1") == "1"
        self.use_lrelu_act = os.environ.get("KLRELU", "1") == "1"

    def mm(self, ps, lhsT, rhs, **kw):
        """matmul, using float32r (full-rate fp32 streaming, ~13-bit
        mantissa) for wide moving operands. Operand tiles must have been
        written through a float32r-rounding producer."""
        if (self.use_f32r and lhsT.dtype == F32 and rhs.dtype == F32
                and rhs.free_size() >= 256):
            lhsT = lhsT.bitcast(F32R)
            rhs = rhs.bitcast(F32R)
        self.nc.tensor.matmul(ps, lhsT, rhs, **kw)

    def rcopy(self, out, in_):
        """ACT copy that rounds to f32r (matmul-operand producer)."""
        o = out.bitcast(F32R) if self.use_f32r else out
        self.nc.scalar.copy(out=o, in_=in_)

    _rr = 0

    def copy_any(self, out, in_, rnd=False):
        """PSUM->SBUF copy, round-robin between DVE and ACT to balance
        engine load. rnd rounds to f32r."""
        if rnd and self.use_f32r:
            out = out.bitcast(F32R)
        Builder._rr += 1
        if Builder._rr % 2 == 0:
            self.nc.vector.tensor_copy(out=out, in_=in_)
        else:
            self.nc.scalar.copy(out=out, in_=in_)

    # ---------- generic helpers ----------

    def transpose_tiles(self, src, dst, rnd=False):
        """dst[i][:, j*128:(j+1)*128] = src[j][:, i*128:(i+1)*128].T
        rnd=True rounds the copy-out to float32r (for matmul operands)."""
        nc = self.nc
        rnd = rnd and self.use_f32r
        for i in range(len(dst)):
            for j in range(len(src)):
                pt = self.pt.tile([P, P], F32, tag="pt", name="pt")
                nc.tensor.transpose(pt, src[j][:, i * P:(i + 1) * P], self.ident)
                self.copy_any(dst[i][:, j * P:(j + 1) * P], pt, rnd=rnd)

    def l2norm_rows(self, src, dst):
        """Row-l2-normalize node-major tiles src -> dst ([128, C] each)."""
        nc = self.nc
        for s, d in zip(src, dst):
            ss = self.col.tile([P, 1], F32, tag="ss", name="ss")
            sq = self.work.tile([P, C], F32, tag="scratch", name="sq")
            nc.scalar.activation(out=sq, in_=s, func=AF.Square, accum_out=ss)
            rn = self.col.tile([P, 1], F32, tag="rn", name="rn")
            nc.scalar.sqrt(out=rn, in_=ss)
            nc.vector.tensor_scalar_max(out=rn, in0=rn, scalar1=EPS)
            nc.vector.reciprocal(out=rn, in_=rn)
            nc.vector.tensor_scalar_mul(out=d, in0=s, scalar1=rn)

    def layernorm_lrelu(self, in_ap, out_ap, g_row, b_row, offload=False,
                        act_lrelu=None):
        """LN over free dim (512) then leaky-relu 0.01. in_ap PSUM/SBUF."""
        nc = self.nc
        stats = self.col.tile([P, 6], F32, tag="bnst", name="bnst")
        nc.vector.bn_stats(out=stats, in_=in_ap)
        mv = self.col.tile([P, 2], F32, tag="bnmv", name="bnmv")
        nc.vector.bn_aggr(out=mv, in_=stats)
        rstd = self.col.tile([P, 1], F32, tag="rstd", name="rstd")
        nc.scalar.activation(out=rstd, in_=mv[:, 1:2], func=AF.Sqrt,
                             bias=self.eps_ln)
        nc.vector.reciprocal(out=rstd, in_=rstd)
        z = self.work.tile([P, C], F32, tag="scratch", name="lnz")
        nc.vector.tensor_scalar(out=z, in0=in_ap, scalar1=mv[:, 0:1],
                                scalar2=rstd, op0=ALU.subtract, op1=ALU.mult)
        nc.vector.tensor_tensor(out=z, in0=z, in1=g_row, op=ALU.mult)
        if act_lrelu is None:
            act_lrelu = self.use_lrelu_act
        if act_lrelu:
            zb2 = self.work.tile([P, C], F32, tag="scratch", name="lnzb")
            nc.vector.tensor_tensor(out=zb2, in0=z, in1=b_row, op=ALU.add)
            nc.scalar.activation(out=out_ap, in_=zb2, func=AF.Lrelu,
                                 alpha=0.01)
        else:
            nc.vector.tensor_tensor(out=z, in0=z, in1=b_row, op=ALU.add)
            z2 = self.work.tile([P, C], F32, tag="scratch", name="lnz2")
            nc.vector.tensor_scalar_mul(out=z2, in0=z, scalar1=0.01)
            nc.vector.tensor_tensor(out=out_ap, in0=z, in1=z2, op=ALU.max)

    def matmul_nt(self, lhsT_slices, rhs_tiles, out_cb, nfree=C):
        """out = sum_k lhsT_slices[k].T @ rhs_tiles[k] -> out_cb(psum)."""
        nc = self.nc
        ps = self.pm.tile([P, nfree], F32, tag="pm", name="pm")
        nk = len(lhsT_slices)
        for k in range(nk):
            self.mm(ps, lhsT_slices[k], rhs_tiles[k],
                    start=(k == 0), stop=(k == nk - 1))
        out_cb(ps)

    def xw_plus(self, xT, wT, out_raw=None, out_biased=None, b_row=None):
        """z = x @ W^T (+ b) from transposed input tiles xT."""
        nc = self.nc
        for m in range(NT):
            def cb(ps, m=m):
                if out_raw is not None:
                    self.copy_any(out_raw[m], ps, rnd=True)
                if out_biased is not None:
                    nc.vector.tensor_tensor(out=out_biased[m], in0=ps,
                                            in1=b_row, op=ALU.add)
            self.matmul_nt(
                [xT[k][:, m * P:(m + 1) * P] for k in range(CT)], wT, cb)


def _load_w_tiles(nc, B, pool, w_dram):
    tiles = []
    for k in range(CT):
        stg = pool.tile([P, C], F32, tag="wstage", name="wstage", bufs=2)
        nc.sync.dma_start(out=stg, in_=w_dram[k * P:(k + 1) * P, :])
        t = pool.tile([P, C], F32, tag="wmat", name="wmat")
        B.rcopy(t, stg)
        tiles.append(t)
    return tiles


def build_nc(phases=("dp", "tc", "bi", "fa")):
    phases = set(phases)
    nc = bass.Bass()

    # ----- DRAM I/O -----
    x_d = nc.dram_tensor("x", [N, C], F32, kind="ExternalInput")
    g_d = nc.dram_tensor("g", [GN, C], F32, kind="ExternalInput")
    pos_d = nc.dram_tensor("pos", [N, POSD], F32, kind="ExternalInput")
    wv_d = nc.dram_tensor("wvec", [5], F32, kind="ExternalInput")

    graph_ops = ["dp", "fa1", "fa2", "fa3"]
    wT_d, b_d, g_ln_d, bln_d = {}, {}, {}, {}
    for o in graph_ops:
        wT_d[o] = nc.dram_tensor(f"{o}_wT", [C, C], F32, kind="ExternalInput")
        b_d[o] = nc.dram_tensor(f"{o}_b", [C], F32, kind="ExternalInput")
        g_ln_d[o] = nc.dram_tensor(f"{o}_g", [C], F32, kind="ExternalInput")
        bln_d[o] = nc.dram_tensor(f"{o}_bln", [C], F32, kind="ExternalInput")
    bi_w1T_d = nc.dram_tensor("bi_w1T", [C, C], F32, kind="ExternalInput")
    bi_w2T_d = nc.dram_tensor("bi_w2T", [M, C], F32, kind="ExternalInput")
    bi_b_d = nc.dram_tensor("bi_b", [C], F32, kind="ExternalInput")
    bi_g_d = nc.dram_tensor("bi_g", [C], F32, kind="ExternalInput")
    bi_bln_d = nc.dram_tensor("bi_bln", [C], F32, kind="ExternalInput")
    tc_cw_d = nc.dram_tensor("tc_cw", [KW, C], F32, kind="ExternalInput")
    tc_cb_d = nc.dram_tensor("tc_cb", [C], F32, kind="ExternalInput")
    tc_g_d = nc.dram_tensor("tc_g", [C], F32, kind="ExternalInput")
    tc_bln_d = nc.dram_tensor("tc_bln", [C], F32, kind="ExternalInput")
    na_wA_d, na_wp_d, na_b_d = {}, {}, {}
    for i in (1, 2, 3):
        na_wA_d[i] = nc.dram_tensor(f"na{i}_wA", [N, 1], F32, kind="ExternalInput")
        na_wp_d[i] = nc.dram_tensor(f"na{i}_wp", [POSD], F32, kind="ExternalInput")
        na_b_d[i] = nc.dram_tensor(f"na{i}_b", [1], F32, kind="ExternalInput")

    y_d = nc.dram_tensor("y", [N, C], F32, kind="ExternalOutput")

    with tile.TileContext(nc) as tcx:
        B = Builder(nc)
        with (
            tcx.tile_pool(name="pers", bufs=1) as pers,
            tcx.tile_pool(name="wpool", bufs=8) as wpool,
            tcx.tile_pool(name="work", bufs=8) as work,
            tcx.tile_pool(name="col", bufs=8) as col,
            tcx.tile_pool(name="pm", bufs=3, space="PSUM") as pm,
            tcx.tile_pool(name="pt", bufs=3, space="PSUM") as pt,
        ):
            B.work, B.col, B.pm, B.pt = work, col, pm, pt

            ident = pers.tile([P, P], F32, tag="ident", name="ident")
            make_identity(nc, ident)
            B.ident = ident
            identb = pers.tile([P, P], mybir.dt.bfloat16, tag="identb",
                               name="identb")
            make_identity(nc, identb)
            eps_ln = pers.tile([P, 1], F32, tag="epsln", name="epsln")
            nc.vector.memset(eps_ln, 1e-5)
            B.eps_ln = eps_ln

            X = [pers.tile([P, C], F32, tag=f"X{m}", name=f"X{m}")
                 for m in range(NT)]
            for m in range(NT):
                nc.sync.dma_start(out=X[m], in_=x_d[m * P:(m + 1) * P, :])

            wvrow = _bcast_row(nc, pers, wv_d[:], 5, tag="wvrow")
            wv = [wvrow[:, i:i + 1] for i in range(5)]

            pos_t = [pers.tile([P, POSD], F32, tag=f"pos{m}", name=f"pos{m}")
                     for m in range(NT)]
            for m in range(NT):
                nc.sync.dma_start(out=pos_t[m], in_=pos_d[m * P:(m + 1) * P, :])

            # branch outputs (consumed by fa1/2/3 later)
            dpout = [pers.tile([P, C], F32, tag=f"dpo{m}", name=f"dpo{m}")
                     for m in range(NT)]
            tcacc = [pers.tile([P, C], F32, tag=f"tca{m}", name=f"tca{m}")
                     for m in range(NT)]
            biout = [pers.tile([P, C], F32, tag=f"bio{m}", name=f"bio{m}")
                     for m in range(NT)]
            out_acc = [pers.tile([P, C], F32, tag=f"oacc{m}", name=f"oacc{m}")
                       for m in range(NT)]
            for m in range(NT):
                nc.vector.tensor_scalar_mul(out=out_acc[m], in0=X[m],
                                            scalar1=wv[1])

            with tcx.tile_pool(name="early", bufs=1) as early:
                # nf, nfT, S shared by dp / tc / bi
                nf = [early.tile([P, C], F32, tag=f"nf{m}", name=f"nf{m}")
                      for m in range(NT)]
                B.l2norm_rows(X, nf)
                nfT = [early.tile([P, N], F32, tag=f"nfT{k}", name=f"nfT{k}")
                       for k in range(CT)]
                B.transpose_tiles(nf, nfT, rnd=True)
                S_sb = [early.tile([P, N], F32, tag=f"S{m}", name=f"S{m}")
                        for m in range(NT)]
                for m in range(NT):
                    B.matmul_nt(
                        [nfT[k][:, m * P:(m + 1) * P] for k in range(CT)],
                        nfT,
                        lambda ps, m=m: B.copy_any(S_sb[m], ps),
                        nfree=N)

                # ===================== diff_prop =====================
                if "dp" not in phases:
                    for m in range(NT):
                        nc.vector.memset(dpout[m], 0.0)
                if "dp" in phases:
                  with tcx.tile_pool(name="dpp", bufs=1) as dpp:
                    xT = [dpp.tile([P, N], F32, tag=f"xT{k}", name=f"xT{k}")
                          for k in range(CT)]
                    B.transpose_tiles(X, xT, rnd=True)
                    zraw = [dpp.tile([P, C], F32, tag=f"zr{m}", name=f"zr{m}")
                            for m in range(NT)]
                    zb = [dpp.tile([P, C], F32, tag=f"zb{m}", name=f"zb{m}")
                          for m in range(NT)]
                    dpb = _bcast_row(nc, dpp, b_d["dp"][:], C, tag="dpb")
                    dpg = _bcast_row(nc, dpp, g_ln_d["dp"][:], C, tag="dpg")
                    dpbl = _bcast_row(nc, dpp, bln_d["dp"][:], C, tag="dpbl")
                    wT = _load_w_tiles(nc, B, wpool, wT_d["dp"])
                    B.xw_plus(xT, wT, out_raw=zraw, out_biased=zb, b_row=dpb)

                    E = [dpp.tile([P, N], F32, tag=f"E{m}", name=f"E{m}")
                         for m in range(NT)]
                    s_cols, recn_cols = [], []
                    for m in range(NT):
                        fs = col.tile([P, 1], F32, tag="dpfs", name="dpfs")
                        # mask diagonal to -inf before exp -> exp gives 0
                        Sd = work.tile([P, N], F32, tag="scratch", name="Sd")
                        nc.gpsimd.affine_select(
                            out=Sd, in_=S_sb[m], compare_op=ALU.not_equal,
                            fill=-1e30, base=m * P, pattern=[[-1, N]],
                            channel_multiplier=1)
                        eout = (E[m].bitcast(F32R) if B.use_f32r else E[m])
                        nc.scalar.activation(out=eout, in_=Sd,
                                             func=AF.Exp, scale=5.0,
                                             accum_out=fs)
                        rec = col.tile([P, 1], F32, tag="dprec", name="dprec")
                        nc.vector.tensor_scalar_max(out=rec, in0=fs, scalar1=EPS)
                        nc.vector.reciprocal(out=rec, in_=rec)
                        s_col = col.tile([P, 1], F32, tag="dps", name="dps")
                        nc.vector.tensor_tensor(out=s_col, in0=fs, in1=rec,
                                                op=ALU.mult)
                        recn = col.tile([P, 1], F32, tag="dprecn", name="dprecn")
                        nc.scalar.mul(out=recn, in_=rec, mul=-1.0)
                        s_cols.append(s_col)
                        recn_cols.append(recn)

                    for m in range(NT):
                        szb = work.tile([P, C], F32, tag="scratch", name="szb")
                        nc.vector.tensor_scalar_mul(out=szb, in0=zb[m],
                                                    scalar1=s_cols[m])

                        def cb(ps, m=m, szb=szb):
                            yt = work.tile([P, C], F32, tag="scratch",
                                           name="dpy")
                            nc.vector.scalar_tensor_tensor(
                                out=yt, in0=ps, scalar=recn_cols[m], in1=szb,
                                op0=ALU.mult, op1=ALU.add)
                            B.layernorm_lrelu(yt, dpout[m], dpg, dpbl)
                        B.matmul_nt(
                            [E[k][:, m * P:(m + 1) * P] for k in range(NT)],
                            zraw, cb)

                # ===== branch 1 (dp -> fa1/na1): overlaps with tc/bi =====
                if "fa" in phases and "dp" in phases:
                    _branch(nc, B, tcx, wpool, work, col, 1, dpout, wT_d,
                            b_d, g_ln_d, bln_d, na_wA_d, na_wp_d, na_b_d,
                            wv, pos_t, out_acc)

                # ===================== temp_conv =====================
                if "tc" not in phases:
                    for m in range(NT):
                        nc.vector.memset(tcacc[m], 0.0)
                if "tc" in phases:
                  with tcx.tile_pool(name="tcp", bufs=1) as tcp:
                    BF16 = mybir.dt.bfloat16
                    # one-hot(argmax per frame) in bf16 (exact 0/1)
                    OH = [tcp.tile([P, N], BF16, tag=f"OH{m}", name=f"OH{m}")
                          for m in range(NT)]
                    for m in range(NT):
                        gmax = col.tile([P, T], F32, tag="gmax", name="gmax")
                        s3 = S_sb[m].rearrange("p (t n) -> p t n", t=T)
                        nc.vector.reduce_max(out=gmax, in_=s3, axis=AX.X)
                        g3 = gmax.rearrange("p (t o) -> p t o", o=1)
                        nc.vector.tensor_tensor(
                            out=OH[m].rearrange("p (t n) -> p t n", t=T),
                            in0=s3, in1=g3.broadcast_to((P, T, NPF)),
                            op=ALU.is_equal)
                    OHT = [tcp.tile([P, N], BF16, tag=f"OHT{m}",
                                    name=f"OHT{m}") for m in range(NT)]
                    for i in range(NT):
                        for j in range(NT):
                            ptb = pt.tile([P, P], BF16, tag="pt", name="ptb")
                            nc.tensor.transpose(
                                ptb, OH[j][:, i * P:(i + 1) * P], identb)
                            nc.vector.tensor_copy(
                                out=OHT[i][:, j * P:(j + 1) * P], in_=ptb)

                    tccb = _bcast_row(nc, tcp, tc_cb_d[:], C, tag="tccb")
                    tcg = _bcast_row(nc, tcp, tc_g_d[:], C, tag="tcg")
                    tcbl = _bcast_row(nc, tcp, tc_bln_d[:], C, tag="tcbl")

                    # Shift-packed operands: row r=(k,j) of column t holds
                    # frame (t+k): OH4/W4 for taps 0-3 (K=128), OH3/W3 for
                    # taps 4-6 (K=96). Repacking via SBUF->SBUF DMA.
                    OH4 = tcp.tile([P, T, N], BF16, tag="OH4", name="OH4")
                    OH3 = tcp.tile([96, T, N], BF16, tag="OH3", name="OH3")
                    W4 = tcp.tile([P, T, C], BF16, tag="W4", name="W4")
                    W3 = tcp.tile([96, T, C], BF16, tag="W3", name="W3")
                    for k in range(KW):
                        dstOH = OH4 if k < 4 else OH3
                        ro = 32 * k if k < 4 else 32 * (k - 4)
                        for t in range(T - k):
                            tp = t + k
                            tl, off = tp // 4, (tp % 4) * NPF
                            nc.sync.dma_start(
                                out=dstOH[ro:ro + NPF, t, :],
                                in_=OHT[tl][off:off + NPF, :])
                    with tcx.tile_pool(name="xwp", bufs=2) as xwp:
                        for k in range(KW):
                            dstW = W4 if k < 4 else W3
                            ro = 32 * k if k < 4 else 32 * (k - 4)
                            cwk = _bcast_row(nc, xwp, tc_cw_d[k, :], C,
                                             tag="cwk")
                            Xwk = [xwp.tile([P, C], BF16, tag="xwk",
                                            name="xwk") for _ in range(NT)]
                            for mm in range(NT):
                                nc.vector.tensor_tensor(
                                    out=Xwk[mm], in0=X[mm], in1=cwk,
                                    op=ALU.mult)
                            for t in range(T - k):
                                tp = t + k
                                tl, off = tp // 4, (tp % 4) * NPF
                                nc.sync.dma_start(
                                    out=dstW[ro:ro + NPF, t, :],
                                    in_=Xwk[tl][off:off + NPF, :])

                    for m in range(NT):
                        for to in range(TO):
                            ps = pm.tile([P, C], F32, tag="pm", name="pm")
                            nc.tensor.matmul(
                                ps, OH4[:, to, m * P:(m + 1) * P],
                                W4[:, to, :], start=True, stop=False)
                            nc.tensor.matmul(
                                ps, OH3[:, to, m * P:(m + 1) * P],
                                W3[:, to, :], start=False, stop=True)
                            yb = work.tile([P, C], F32, tag="scratch",
                                           name="tcyb")
                            nc.vector.tensor_tensor(out=yb, in0=ps, in1=tccb,
                                                    op=ALU.add)
                            zt = work.tile([P, C], F32, tag="scratch",
                                           name="tczt")
                            # ACT-free inner loop (bar the [128,1] sqrt):
                            # avoids activation-table thrash
                            B.layernorm_lrelu(yb, zt, tcg, tcbl,
                                              act_lrelu=False)
                            if to == 0:
                                nc.scalar.copy(out=tcacc[m], in_=zt)
                            else:
                                nc.vector.tensor_tensor(out=tcacc[m],
                                                        in0=tcacc[m], in1=zt,
                                                        op=ALU.add)
                        nc.scalar.mul(out=tcacc[m], in_=tcacc[m], mul=1.0 / TO)

                # ===== branch 2 (tc -> fa2/na2): overlaps with bi =====
                if "fa" in phases and "tc" in phases:
                    _branch(nc, B, tcx, wpool, work, col, 2, tcacc, wT_d,
                            b_d, g_ln_d, bln_d, na_wA_d, na_wp_d, na_b_d,
                            wv, pos_t, out_acc)

                # ===================== back_incor ====================
                if "bi" not in phases:
                    for m in range(NT):
                        nc.vector.memset(biout[m], 0.0)
                if "bi" in phases:
                  with tcx.tile_pool(name="bip", bufs=1) as bip:
                    Gt = [bip.tile([P, C], F32, tag=f"G{i}", name=f"G{i}")
                          for i in range(GT_TILES)]
                    nc.gpsimd.memset(Gt[GT_TILES - 1], 0.0)
                    for i in range(GT_TILES):
                        r0, r1 = i * P, min((i + 1) * P, GN)
                        nc.sync.dma_start(out=Gt[i][:r1 - r0, :],
                                          in_=g_d[r0:r1, :])
                    nfg = [bip.tile([P, C], F32, tag=f"nfg{i}", name=f"nfg{i}")
                           for i in range(GT_TILES)]
                    B.l2norm_rows(Gt, nfg)
                    GNP = GT_TILES * P
                    nfgT = [bip.tile([P, GNP], F32, tag=f"nfgT{k}",
                                     name=f"nfgT{k}") for k in range(CT)]
                    B.transpose_tiles(nfg, nfgT, rnd=True)

                    u_stack = [bip.tile([P, C], F32, tag=f"bu{m}",
                                        name=f"bu{m}") for m in range(NT)]
                    R = [bip.tile([P, M], F32, tag=f"bR{m}", name=f"bR{m}")
                         for m in range(NT)]
                    with tcx.tile_pool(name="gpp", bufs=3) as gpp:
                        for t in range(T):
                            mtile, moff = t // 4, (t % 4) * NPF
                            gpad = gpp.tile([M, C], F32, tag="gpad",
                                            name="gpad")
                            nc.sync.dma_start(out=gpad,
                                              in_=g_d[t * M:(t + 1) * M, :])
                            # Araw_t [32, 49] (raw, for concat path)
                            psA = pt.tile([P, M], F32, tag="pt", name="psA")
                            for k in range(CT):
                                B.mm(psA[:NPF, :],
                                     nfT[k][:, t * NPF:(t + 1) * NPF],
                                     nfgT[k][:, t * M:(t + 1) * M],
                                     start=(k == 0), stop=(k == CT - 1))
                            fs = col.tile([P, 1], F32, tag="bifs", name="bifs")
                            scr = work.tile([P, M], F32, tag="biscr",
                                            name="biscr")
                            nc.scalar.activation(out=scr[:NPF, :],
                                                 in_=psA[:NPF, :],
                                                 func=AF.Exp, scale=5.0,
                                                 accum_out=fs[:NPF, :])
                            nc.scalar.copy(out=R[mtile][moff:moff + NPF, :],
                                           in_=psA[:NPF, :])
                            rs = col.tile([P, 1], F32, tag="birs", name="birs")
                            nc.vector.reciprocal(out=rs[:NPF, :],
                                                 in_=fs[:NPF, :])
                            # ArawT_t [49, 32] -> FT = exp(5 ArawT)
                            psAT = pt.tile([P, NPF], F32, tag="pt", name="psAT")
                            for k in range(CT):
                                B.mm(psAT[:M, :],
                                     nfgT[k][:, t * M:(t + 1) * M],
                                     nfT[k][:, t * NPF:(t + 1) * NPF],
                                     start=(k == 0), stop=(k == CT - 1))
                            FT = work.tile([P, NPF], F32, tag="biFT",
                                           name="biFT")
                            fto = (FT[:M, :].bitcast(F32R) if B.use_f32r
                                   else FT[:M, :])
                            nc.scalar.activation(out=fto, in_=psAT[:M, :],
                                                 func=AF.Exp, scale=5.0)
                            gpr = gpp.tile([M, C], F32, tag="gpr",
                                           name="gpr")
                            B.rcopy(gpr, gpad)
                            # u_t = rs * (FT.T @ gf_t)
                            psu = pm.tile([P, C], F32, tag="pm", name="psu")
                            B.mm(psu[:NPF, :], FT[:M, :], gpr,
                                 start=True, stop=True)
                            nc.vector.tensor_scalar_mul(
                                out=u_stack[mtile][moff:moff + NPF, :],
                                in0=psu[:NPF, :], scalar1=rs[:NPF, :])

                    uT = [bip.tile([P, N], F32, tag=f"buT{k}", name=f"buT{k}")
                          for k in range(CT)]
                    B.transpose_tiles(u_stack, uT, rnd=True)
                    RT = bip.tile([M, N], F32, tag="bRT", name="bRT")
                    for m in range(NT):
                        ptr = pt.tile([P, P], F32, tag="pt", name="ptr")
                        nc.tensor.transpose(ptr[:M, :], R[m], ident)
                        B.rcopy(RT[:, m * P:(m + 1) * P], ptr[:M, :])

                    w1T = _load_w_tiles(nc, B, wpool, bi_w1T_d)
                    w2s = wpool.tile([M, C], F32, tag="w2s", name="w2s",
                                     bufs=1)
                    nc.sync.dma_start(out=w2s, in_=bi_w2T_d[:, :])
                    w2T = wpool.tile([M, C], F32, tag="w2", name="w2", bufs=1)
                    B.rcopy(w2T, w2s)
                    bib = _bcast_row(nc, bip, bi_b_d[:], C, tag="bib")
                    big_ = _bcast_row(nc, bip, bi_g_d[:], C, tag="big")
                    bibl = _bcast_row(nc, bip, bi_bln_d[:], C, tag="bibl")
                    for m in range(NT):
                        ps = pm.tile([P, C], F32, tag="pm", name="pm")
                        for k in range(CT):
                            B.mm(ps, uT[k][:, m * P:(m + 1) * P],
                                 w1T[k], start=(k == 0), stop=False)
                        B.mm(ps, RT[:, m * P:(m + 1) * P], w2T,
                             start=False, stop=True)
                        yb = work.tile([P, C], F32, tag="scratch", name="biyb")
                        nc.vector.tensor_tensor(out=yb, in0=ps, in1=bib,
                                                op=ALU.add)
                        B.layernorm_lrelu(yb, biout[m], big_, bibl)

            # ============== remaining branch (bi -> fa3/na3) ==============
            if "fa" in phases:
                _branch(nc, B, tcx, wpool, work, col, 3, biout, wT_d, b_d,
                        g_ln_d, bln_d, na_wA_d, na_wp_d, na_b_d, wv, pos_t,
                        out_acc)

            for m in range(NT):
                nc.sync.dma_start(out=y_d[m * P:(m + 1) * P, :],
                                  in_=out_acc[m])

    split_multiwaits(nc)
    return nc



def _branch(nc, B, tcx, wpool, work, col, idx, xin, wT_d, b_d, g_ln_d,
            bln_d, na_wA_d, na_wp_d, na_b_d, wv, pos_t, out_acc):
    fa = f"fa{idx}"
    with tcx.tile_pool(name=f"br{idx}", bufs=1) as brp:
        faout = _feat_aggr(nc, B, brp, wpool, work, col, xin,
                           wT_d[fa], b_d[fa], g_ln_d[fa], bln_d[fa])
        _node_att(nc, B, brp, work, col, faout, pos_t,
                  na_wA_d[idx], na_wp_d[idx], na_b_d[idx],
                  wv[idx + 1], out_acc)


def _feat_aggr(nc, B, brp, wpool, work, col, xin, wT_dram, b_dram,
               g_dram, bln_dram):
    """y = lrelu(LN(softmax(5 nf nf^T) @ (x W^T + b)))"""
    xinT = [brp.tile([P, N], F32, tag=f"fxT{k}", name=f"fxT{k}")
            for k in range(CT)]
    B.transpose_tiles(xin, xinT, rnd=True)
    nfp = [brp.tile([P, C], F32, tag=f"fnf{m}", name=f"fnf{m}")
           for m in range(NT)]
    B.l2norm_rows(xin, nfp)
    nfpT = [brp.tile([P, N], F32, tag=f"fnfT{k}", name=f"fnfT{k}")
            for k in range(CT)]
    B.transpose_tiles(nfp, nfpT, rnd=True)

    wT = _load_w_tiles(nc, B, wpool, wT_dram)
    fab = _bcast_row(nc, brp, b_dram[:], C, tag="fab")
    fag = _bcast_row(nc, brp, g_dram[:], C, tag="fag")
    fabl = _bcast_row(nc, brp, bln_dram[:], C, tag="fabl")
    # bias moves out of the matmul: softmax rows sum to 1, so
    # A@(z + 1 b^T) = A@z + b  -> apply b after the aggregation.
    z = [brp.tile([P, C], F32, tag=f"fz{m}", name=f"fz{m}")
         for m in range(NT)]
    B.xw_plus(xinT, wT, out_raw=z)

    F = [brp.tile([P, N], F32, tag=f"fF{m}", name=f"fF{m}") for m in range(NT)]
    rs_cols = []
    for m in range(NT):
        fs = col.tile([P, 1], F32, tag="fafs", name="fafs")

        def cb(ps, m=m, fs=fs):
            fo = F[m].bitcast(F32R) if B.use_f32r else F[m]
            nc.scalar.activation(out=fo, in_=ps, func=AF.Exp, scale=5.0,
                                 accum_out=fs)
        B.matmul_nt([nfpT[k][:, m * P:(m + 1) * P] for k in range(CT)],
                    nfpT, cb, nfree=N)
        rs = col.tile([P, 1], F32, tag="fars", name="fars")
        nc.vector.reciprocal(out=rs, in_=fs)
        rs_cols.append(rs)

    out = []
    for m in range(NT):
        o = brp.tile([P, C], F32, tag=f"fo{m}", name=f"fo{m}")

        def cb(ps, m=m, o=o):
            yt = work.tile([P, C], F32, tag="scratch", name="fayt")
            nc.vector.scalar_tensor_tensor(out=yt, in0=ps,
                                           scalar=rs_cols[m], in1=fab,
                                           op0=ALU.mult, op1=ALU.add)
            B.layernorm_lrelu(yt, o, fag, fabl)
        B.matmul_nt([F[k][:, m * P:(m + 1) * P] for k in range(NT)], z, cb)
        out.append(o)
    return out


def _node_att(nc, B, brp, work, col, xin, pos_t, wA_dram, wp_dram, b_dram,
              wv_col, out_acc):
    """out_acc += wv * (xin * sigmoid(nf@(nf^T@wA) + pos@wp + b))
    A@wA == nf @ (nf^T @ wA): two PE matvecs, A never materialized."""
    nfp = [brp.tile([P, C], F32, tag=f"nnf{m}", name=f"nnf{m}")
           for m in range(NT)]
    B.l2norm_rows(xin, nfp)
    nfpT = [brp.tile([P, N], F32, tag=f"nnfT{k}", name=f"nnfT{k}")
            for k in range(CT)]
    B.transpose_tiles(nfp, nfpT)

    wA = []
    for i in range(NT):
        t = brp.tile([P, 1], F32, tag=f"nwA{i}", name=f"nwA{i}")
        nc.sync.dma_start(out=t, in_=wA_dram[i * P:(i + 1) * P, :])
        wA.append(t)
    wp_row = _bcast_row(nc, brp, wp_dram[:], POSD, tag="nwp")
    b_row = _bcast_row(nc, brp, b_dram[:], 1, tag="nb")

    # v = nf^T @ wA (channel-major column, CT pieces [128, 1])
    v = []
    for c in range(CT):
        pv = B.pt.tile([P, 1], F32, tag="pt", name="pv")
        for m in range(NT):
            B.mm(pv, nfp[m][:, c * P:(c + 1) * P], wA[m],
                 start=(m == 0), stop=(m == NT - 1))
        vc = col.tile([P, 1], F32, tag="nav", name="nav")
        nc.scalar.copy(out=vc, in_=pv)
        v.append(vc)

    for m in range(NT):
        pa = B.pt.tile([P, 1], F32, tag="pt", name="pa")
        for k in range(CT):
            B.mm(pa, nfpT[k][:, m * P:(m + 1) * P], v[k],
                 start=(k == 0), stop=(k == CT - 1))
        sp = work.tile([P, POSD], F32, tag="naps", name="naps")
        nc.vector.tensor_tensor(out=sp, in0=pos_t[m], in1=wp_row, op=ALU.mult)
        pw = col.tile([P, 1], F32, tag="napw", name="napw")
        nc.vector.reduce_sum(out=pw, in_=sp, axis=AX.X)
        att = col.tile([P, 1], F32, tag="naatt", name="naatt")
        nc.vector.tensor_tensor(out=att, in0=pa, in1=pw, op=ALU.add)
        nc.scalar.activation(out=pw, in_=att, func=AF.Sigmoid,
                             bias=b_row, scale=1.0)
        contrib = work.tile([P, C], F32, tag="scratch", name="nact")
        nc.vector.tensor_scalar(out=contrib, in0=xin[m], scalar1=pw,
                                scalar2=wv_col, op0=ALU.mult, op1=ALU.mult)
        nc.gpsimd.tensor_tensor(out=out_acc[m], in0=out_acc[m], in1=contrib,
                                op=ALU.add)


# =====================================================================
# host wrapper
# =====================================================================

_NC_CACHE = {}


def _get_nc():
    if "nc" not in _NC_CACHE:
        _NC_CACHE["nc"] = build_nc()
    return _NC_CACHE["nc"]


def make_in_maps(local_feat, global_feat, pos, weights, params):
    def ap(v):
        return np.ascontiguousarray(np.asarray(v, dtype=np.float32))

    shared = {"wvec": ap(weights)}
    for o in ("dp", "fa1", "fa2", "fa3"):
        p = params[o]
        assert np.allclose(np.diagonal(ap(p["adj_w"])), 1.0), "adj diag != 1"
        shared[f"{o}_wT"] = ap(np.asarray(p["aff_w"]).T)
        shared[f"{o}_b"] = ap(p["aff_b"])
        shared[f"{o}_g"] = ap(p["ln_g"])
        shared[f"{o}_bln"] = ap(p["ln_b"])
    pbi = params["bi"]
    assert np.allclose(np.diagonal(ap(pbi["adj_w"])), 1.0)
    wbi = ap(pbi["aff_w"])          # [512, 561]
    shared["bi_w1T"] = ap(wbi[:, :C].T)
    shared["bi_w2T"] = ap(wbi[:, C:].T)
    shared["bi_b"] = ap(pbi["aff_b"])
    shared["bi_g"] = ap(pbi["ln_g"])
    shared["bi_bln"] = ap(pbi["ln_b"])
    ptc = params["tc"]
    assert np.allclose(np.diagonal(ap(ptc["adj_w"])), 1.0)
    shared["tc_cw"] = ap(np.asarray(ptc["conv_w"])[:, 0, :].T)   # [7, 512]
    shared["tc_cb"] = ap(ptc["conv_b"])
    shared["tc_g"] = ap(ptc["ln_g"])
    shared["tc_bln"] = ap(ptc["ln_b"])
    for i, na in ((1, "na1"), (2, "na2"), (3, "na3")):
        att_w = ap(params[na]["att_w"])[0]      # [521]
        shared[f"na{i}_wA"] = ap(att_w[:N][:, None])
        shared[f"na{i}_wp"] = ap(att_w[N:])
        shared[f"na{i}_b"] = ap(params[na]["att_b"])

    in_maps = []
    for b in range(local_feat.shape[0]):
        m = dict(shared)
        m["x"] = ap(local_feat[b].reshape(N, C))
        m["g"] = ap(global_feat[b].reshape(GN, C))
        m["pos"] = ap(pos[b].reshape(N, POSD))
        in_maps.append(m)
    return in_maps


def kernel(local_feat, global_feat, pos, weights, params):
    from concourse.bass_utils import run_bass_kernel_spmd

    local_feat = np.asarray(local_feat, dtype=np.float32)
    global_feat = np.asarray(global_feat, dtype=np.float32)
    pos = np.asarray(pos, dtype=np.float32)
    weights = np.asarray(weights, dtype=np.float32)

    in_maps = make_in_maps(local_feat, global_feat, pos, weights, params)
    nc = _get_nc()
    res = run_bass_kernel_spmd(nc, in_maps, core_ids=list(range(8)))
    out = np.stack([res.results[b]["y"].reshape(T, NPF, C)
                    for b in range(8)])
    return out.astype(np.float32)


# revision 31
# speedup vs baseline: 1.4673x; 1.2763x over previous
"""Trainium2 Bass kernel for nn_MixedOp (gnn_message_passing).

Strategy: data-parallel over batch (B=8 -> 1 sample/core). Everything on
device except zero-FLOP host marshaling (reshapes / weight transposes /
diagonal extraction; adj_w is the identity so diag(adj_w) == 1).

Math restructuring (exact up to FP reassociation):
- softmax(5A)@Z = (1/rowsum(exp(5S))) * (exp(5S) @ Z) with S = nf@nf^T
  symmetric -> exp(5S) symmetric -> its tiles usable as matmul lhsT
  directly (no attention-matrix transpose).
- diff_prop: y = s*(xW+b) - rec*(E@(xW)) with E = exp(5S) diag-zeroed,
  rec = 1/max(rowsum E, eps), s = rowsum(E)*rec.
- node_att: A@wA = nf @ (nf^T @ wA): two matvecs, A never materialized.
- temp_conv: one-hot(argmax) per 32-node frame; gather + depthwise conv
  folded into PSUM-accumulated matmuls: y_to = sum_k OH_{to+k}^T-slice @
  (X * w_k)_{to+k}-rows.
"""

import sys

for _p in ("/opt/trn_rl_repo",):
    if _p not in sys.path:
        sys.path.insert(0, _p)

import numpy as np

import concourse.bass as bass
import concourse.tile as tile
from concourse import mybir
from concourse.masks import make_identity

F32 = mybir.dt.float32
AF = mybir.ActivationFunctionType
ALU = mybir.AluOpType
AX = mybir.AxisListType

P = 128
N = 512          # nodes per sample (T*NODES)
C = 512          # channels
NT = N // P      # node tiles
CT = C // P      # channel tiles
T = 16           # frames
NPF = 32         # nodes per frame
M = 49           # global nodes per frame
GN = T * M       # 784 global rows
GT_TILES = (GN + P - 1) // P  # 7
KW = 7           # conv taps
TO = T - KW + 1  # 10 conv outputs
POSD = 9
EPS = 1e-12


def split_multiwaits(nc):
    """This walrus build accepts at most ONE sync-wait per instruction;
    Tile emits several. Hoist all but the last onto same-engine NOPs
    placed immediately before the instruction (same blocking semantics;
    each engine's program order is a topological order of the dep graph,
    so the hoisted waits cannot deadlock)."""
    n = 0
    for fn in nc.m.functions:
        for bb in fn.blocks:
            new_list = []
            for inst in bb.instructions:
                si = inst.sync_info
                if si is not None and si.on_wait and len(si.on_wait) > 1:
                    waits = list(si.on_wait)
                    for w in waits[:-1]:
                        nop = mybir.InstNoOp(
                            name=f"I-splitw-{n}",
                            engine=inst.engine,
                            ins=[],
                            outs=[],
                            sync_info=mybir.SyncInfo(on_wait=[w], on_update=[]),
                        )
                        n += 1
                        nc.register_instruction(nop)
                        new_list.append(nop)
                    inst.sync_info = mybir.SyncInfo(
                        on_wait=[waits[-1]], on_update=si.on_update
                    )
                new_list.append(inst)
            bb.instructions[:] = new_list
    return n


def _bcast_row(nc, pool, dram_ap, length, tag="brow"):
    """DMA a [length] DRAM vector to SBUF broadcast across 128 partitions."""
    t = pool.tile([P, length], F32, tag=tag, name=tag)
    src = bass.AP(tensor=dram_ap.tensor, offset=dram_ap.offset,
                  ap=[[0, P], [1, length]])
    nc.sync.dma_start(out=t, in_=src)
    return t


F32R = mybir.dt.float32r


class Builder:
    def __init__(self, nc):
        self.nc = nc
        import os
        self.use_f32r = os.environ.get("KF32R", "1") == "1"
        self.use_lrelu_act = os.environ.get("KLRELU", "1") == "1"

    def mm(self, ps, lhsT, rhs, **kw):
        """matmul, using float32r (full-rate fp32 streaming, ~13-bit
        mantissa) for wide moving operands. Operand tiles must have been
        written through a float32r-rounding producer."""
        if (self.use_f32r and lhsT.dtype == F32 and rhs.dtype == F32
                and rhs.free_size() >= 256):
            lhsT = lhsT.bitcast(F32R)
            rhs = rhs.bitcast(F32R)
        self.nc.tensor.matmul(ps, lhsT, rhs, **kw)

    def rcopy(self, out, in_):
        """ACT copy that rounds to f32r (matmul-operand producer)."""
        o = out.bitcast(F32R) if self.use_f32r else out
        self.nc.scalar.copy(out=o, in_=in_)

    _rr = 0

    def copy_any(self, out, in_, rnd=False):
        """PSUM->SBUF copy, round-robin between DVE and ACT to balance
        engine load. rnd rounds to f32r."""
        if rnd and self.use_f32r:
            out = out.bitcast(F32R)
        Builder._rr += 1
        if Builder._rr % 2 == 0:
            self.nc.vector.tensor_copy(out=out, in_=in_)
        else:
            self.nc.scalar.copy(out=out, in_=in_)

    # ---------- generic helpers ----------

    def transpose_tiles(self, src, dst, rnd=False):
        """dst[i][:, j*128:(j+1)*128] = src[j][:, i*128:(i+1)*128].T
        rnd=True rounds the copy-out to float32r (for matmul operands)."""
        nc = self.nc
        rnd = rnd and self.use_f32r
        for i in range(len(dst)):
            for j in range(len(src)):
                pt = self.pt.tile([P, P], F32, tag="pt", name="pt")
                nc.tensor.transpose(pt, src[j][:, i * P:(i + 1) * P], self.ident)
                self.copy_any(dst[i][:, j * P:(j + 1) * P], pt, rnd=rnd)

    def l2norm_rows(self, src, dst):
        """Row-l2-normalize node-major tiles src -> dst ([128, C] each)."""
        nc = self.nc
        for s, d in zip(src, dst):
            ss = self.col.tile([P, 1], F32, tag="ss", name="ss")
            sq = self.work.tile([P, C], F32, tag="scratch", name="sq")
            nc.scalar.activation(out=sq, in_=s, func=AF.Square, accum_out=ss)
            rn = self.col.tile([P, 1], F32, tag="rn", name="rn")
            nc.scalar.sqrt(out=rn, in_=ss)
            nc.vector.tensor_scalar_max(out=rn, in0=rn, scalar1=EPS)
            nc.vector.reciprocal(out=rn, in_=rn)
            nc.vector.tensor_scalar_mul(out=d, in0=s, scalar1=rn)

    def layernorm_lrelu(self, in_ap, out_ap, g_row, b_row, offload=False,
                        act_lrelu=None):
        """LN over free dim then leaky-relu 0.01 (streaming form)."""
        self.ln_batch([in_ap], [out_ap], g_row, b_row, act_lrelu=act_lrelu)

    def ln_batch(self, ins, outs, g_row, b_row, act_lrelu=None):
        """LayerNorm + lrelu over a batch of [128, C] SBUF tiles with the
        sqrt/reciprocal vectorized across the batch and activation calls
        grouped (one table load)."""
        nc = self.nc
        if act_lrelu is None:
            act_lrelu = self.use_lrelu_act
        n = len(ins)
        var_n = self.col.tile([P, n], F32, tag="varn", name="varn", bufs=4)
        mvs = []
        for i, a in enumerate(ins):
            stats = self.col.tile([P, 6], F32, tag="bnst", name="bnst")
            nc.vector.bn_stats(out=stats, in_=a)
            mv = self.col.tile([P, 2], F32, tag="bnmv", name="bnmv")
            nc.vector.bn_aggr(out=mv, in_=stats)
            mvs.append(mv)
            if n > 1:
                nc.vector.tensor_copy(out=var_n[:, i:i + 1], in_=mv[:, 1:2])
        rstd_n = self.col.tile([P, n], F32, tag="rstdn", name="rstdn", bufs=4)
        nc.scalar.activation(out=rstd_n,
                             in_=var_n if n > 1 else mvs[0][:, 1:2],
                             func=AF.Sqrt, bias=self.eps_ln)
        nc.vector.reciprocal(out=rstd_n, in_=rstd_n)
        zs = []
        for i, a in enumerate(ins):
            z1 = self.work.tile([P, C], F32, tag="scratch", name="lnz1")
            nc.vector.scalar_tensor_tensor(out=z1, in0=a,
                                           scalar=mvs[i][:, 0:1], in1=g_row,
                                           op0=ALU.subtract, op1=ALU.mult)
            z = self.work.tile([P, C], F32, tag="scratch", name="lnz")
            nc.vector.scalar_tensor_tensor(out=z, in0=z1,
                                           scalar=rstd_n[:, i:i + 1],
                                           in1=b_row,
                                           op0=ALU.mult, op1=ALU.add)
            zs.append(z)
        for i, z in enumerate(zs):
            if act_lrelu:
                nc.scalar.activation(out=outs[i], in_=z, func=AF.Lrelu,
                                     alpha=0.01)
            else:
                z2 = self.work.tile([P, C], F32, tag="scratch", name="lnz2")
                nc.vector.tensor_scalar_mul(out=z2, in0=z, scalar1=0.01)
                nc.vector.tensor_tensor(out=outs[i], in0=z, in1=z2,
                                        op=ALU.max)

    def matmul_nt(self, lhsT_slices, rhs_tiles, out_cb, nfree=C):
        """out = sum_k lhsT_slices[k].T @ rhs_tiles[k] -> out_cb(psum)."""
        nc = self.nc
        ps = self.pm.tile([P, nfree], F32, tag="pm", name="pm")
        nk = len(lhsT_slices)
        for k in range(nk):
            self.mm(ps, lhsT_slices[k], rhs_tiles[k],
                    start=(k == 0), stop=(k == nk - 1))
        out_cb(ps)

    def xw_plus(self, xT, wT, out_raw=None, out_biased=None, b_row=None):
        """z = x @ W^T (+ b) from transposed input tiles xT."""
        nc = self.nc
        for m in range(NT):
            def cb(ps, m=m):
                if out_raw is not None:
                    self.copy_any(out_raw[m], ps, rnd=True)
                if out_biased is not None:
                    nc.vector.tensor_tensor(out=out_biased[m], in0=ps,
                                            in1=b_row, op=ALU.add)
            self.matmul_nt(
                [xT[k][:, m * P:(m + 1) * P] for k in range(CT)], wT, cb)


def _load_w_tiles(nc, B, pool, w_dram):
    tiles = []
    for k in range(CT):
        stg = pool.tile([P, C], F32, tag="wstage", name="wstage", bufs=2)
        nc.sync.dma_start(out=stg, in_=w_dram[k * P:(k + 1) * P, :])
        t = pool.tile([P, C], F32, tag="wmat", name="wmat")
        B.rcopy(t, stg)
        tiles.append(t)
    return tiles


def build_nc(phases=("dp", "tc", "bi", "fa")):
    phases = set(phases)
    nc = bass.Bass()

    # ----- DRAM I/O -----
    x_d = nc.dram_tensor("x", [N, C], F32, kind="ExternalInput")
    g_d = nc.dram_tensor("g", [GN, C], F32, kind="ExternalInput")
    pos_d = nc.dram_tensor("pos", [N, POSD], F32, kind="ExternalInput")
    wv_d = nc.dram_tensor("wvec", [5], F32, kind="ExternalInput")

    graph_ops = ["dp", "fa1", "fa2", "fa3"]
    wT_d, b_d, g_ln_d, bln_d = {}, {}, {}, {}
    for o in graph_ops:
        wT_d[o] = nc.dram_tensor(f"{o}_wT", [C, C], F32, kind="ExternalInput")
        b_d[o] = nc.dram_tensor(f"{o}_b", [C], F32, kind="ExternalInput")
        g_ln_d[o] = nc.dram_tensor(f"{o}_g", [C], F32, kind="ExternalInput")
        bln_d[o] = nc.dram_tensor(f"{o}_bln", [C], F32, kind="ExternalInput")
    bi_w1T_d = nc.dram_tensor("bi_w1T", [C, C], F32, kind="ExternalInput")
    bi_w2T_d = nc.dram_tensor("bi_w2T", [M, C], F32, kind="ExternalInput")
    bi_b_d = nc.dram_tensor("bi_b", [C], F32, kind="ExternalInput")
    bi_g_d = nc.dram_tensor("bi_g", [C], F32, kind="ExternalInput")
    bi_bln_d = nc.dram_tensor("bi_bln", [C], F32, kind="ExternalInput")
    tc_cw_d = nc.dram_tensor("tc_cw", [KW, C], F32, kind="ExternalInput")
    tc_cbh_d = nc.dram_tensor("tc_cbh", [C], mybir.dt.bfloat16,
                              kind="ExternalInput")
    tc_g_d = nc.dram_tensor("tc_g", [C], F32, kind="ExternalInput")
    tc_bln_d = nc.dram_tensor("tc_bln", [C], F32, kind="ExternalInput")
    na_wA_d, na_wp_d, na_b_d = {}, {}, {}
    for i in (1, 2, 3):
        na_wA_d[i] = nc.dram_tensor(f"na{i}_wA", [N, 1], F32, kind="ExternalInput")
        na_wp_d[i] = nc.dram_tensor(f"na{i}_wp", [POSD], F32, kind="ExternalInput")
        na_b_d[i] = nc.dram_tensor(f"na{i}_b", [1], F32, kind="ExternalInput")

    y_d = nc.dram_tensor("y", [N, C], F32, kind="ExternalOutput")

    with tile.TileContext(nc) as tcx:
        B = Builder(nc)
        with (
            tcx.tile_pool(name="pers", bufs=1) as pers,
            tcx.tile_pool(name="wpool", bufs=8) as wpool,
            tcx.tile_pool(name="work", bufs=8) as work,
            tcx.tile_pool(name="col", bufs=8) as col,
            tcx.tile_pool(name="pm", bufs=3, space="PSUM") as pm,
            tcx.tile_pool(name="pt", bufs=3, space="PSUM") as pt,
        ):
            B.work, B.col, B.pm, B.pt = work, col, pm, pt

            ident = pers.tile([P, P], F32, tag="ident", name="ident")
            make_identity(nc, ident)
            B.ident = ident
            identb = pers.tile([P, P], mybir.dt.bfloat16, tag="identb",
                               name="identb")
            make_identity(nc, identb)
            eps_ln = pers.tile([P, 1], F32, tag="epsln", name="epsln")
            nc.vector.memset(eps_ln, 1e-5)
            B.eps_ln = eps_ln

            X = [pers.tile([P, C], F32, tag=f"X{m}", name=f"X{m}")
                 for m in range(NT)]
            for m in range(NT):
                nc.sync.dma_start(out=X[m], in_=x_d[m * P:(m + 1) * P, :])

            wvrow = _bcast_row(nc, pers, wv_d[:], 5, tag="wvrow")
            wv = [wvrow[:, i:i + 1] for i in range(5)]

            pos_t = [pers.tile([P, POSD], F32, tag=f"pos{m}", name=f"pos{m}")
                     for m in range(NT)]
            for m in range(NT):
                nc.sync.dma_start(out=pos_t[m], in_=pos_d[m * P:(m + 1) * P, :])

            # branch outputs (consumed by fa1/2/3 later)
            dpout = [pers.tile([P, C], F32, tag=f"dpo{m}", name=f"dpo{m}")
                     for m in range(NT)]
            tcacc = [pers.tile([P, C], F32, tag=f"tca{m}", name=f"tca{m}")
                     for m in range(NT)]
            biout = [pers.tile([P, C], F32, tag=f"bio{m}", name=f"bio{m}")
                     for m in range(NT)]
            out_acc = [pers.tile([P, C], F32, tag=f"oacc{m}", name=f"oacc{m}")
                       for m in range(NT)]
            for m in range(NT):
                nc.vector.tensor_scalar_mul(out=out_acc[m], in0=X[m],
                                            scalar1=wv[1])

            with tcx.tile_pool(name="early", bufs=1) as early:
                # nf, nfT, S shared by dp / tc / bi
                nf = [early.tile([P, C], F32, tag=f"nf{m}", name=f"nf{m}")
                      for m in range(NT)]
                B.l2norm_rows(X, nf)
                nfT = [early.tile([P, N], F32, tag=f"nfT{k}", name=f"nfT{k}")
                       for k in range(CT)]
                B.transpose_tiles(nf, nfT, rnd=True)
                S_sb = [early.tile([P, N], F32, tag=f"S{m}", name=f"S{m}")
                        for m in range(NT)]
                for m in range(NT):
                    B.matmul_nt(
                        [nfT[k][:, m * P:(m + 1) * P] for k in range(CT)],
                        nfT,
                        lambda ps, m=m: B.copy_any(S_sb[m], ps),
                        nfree=N)

                # ===================== diff_prop =====================
                if "dp" not in phases:
                    for m in range(NT):
                        nc.vector.memset(dpout[m], 0.0)
                if "dp" in phases:
                  with tcx.tile_pool(name="dpp", bufs=1) as dpp:
                    xT = [dpp.tile([P, N], F32, tag=f"xT{k}", name=f"xT{k}")
                          for k in range(CT)]
                    B.transpose_tiles(X, xT, rnd=True)
                    zraw = [dpp.tile([P, C], F32, tag=f"zr{m}", name=f"zr{m}")
                            for m in range(NT)]
                    zb = [dpp.tile([P, C], F32, tag=f"zb{m}", name=f"zb{m}")
                          for m in range(NT)]
                    dpb = _bcast_row(nc, dpp, b_d["dp"][:], C, tag="dpb")
                    dpg = _bcast_row(nc, dpp, g_ln_d["dp"][:], C, tag="dpg")
                    dpbl = _bcast_row(nc, dpp, bln_d["dp"][:], C, tag="dpbl")
                    wT = _load_w_tiles(nc, B, wpool, wT_d["dp"])
                    B.xw_plus(xT, wT, out_raw=zraw, out_biased=zb, b_row=dpb)

                    E = [dpp.tile([P, N], F32, tag=f"E{m}", name=f"E{m}")
                         for m in range(NT)]
                    s_cols, recn_cols = [], []
                    for m in range(NT):
                        fs = col.tile([P, 1], F32, tag="dpfs", name="dpfs")
                        # mask diagonal to -inf before exp -> exp gives 0
                        Sd = work.tile([P, N], F32, tag="scratch", name="Sd")
                        nc.gpsimd.affine_select(
                            out=Sd, in_=S_sb[m], compare_op=ALU.not_equal,
                            fill=-1e30, base=m * P, pattern=[[-1, N]],
                            channel_multiplier=1)
                        eout = (E[m].bitcast(F32R) if B.use_f32r else E[m])
                        nc.scalar.activation(out=eout, in_=Sd,
                                             func=AF.Exp, scale=5.0,
                                             accum_out=fs)
                        rec = col.tile([P, 1], F32, tag="dprec", name="dprec")
                        nc.vector.tensor_scalar_max(out=rec, in0=fs, scalar1=EPS)
                        nc.vector.reciprocal(out=rec, in_=rec)
                        s_col = col.tile([P, 1], F32, tag="dps", name="dps")
                        nc.vector.tensor_tensor(out=s_col, in0=fs, in1=rec,
                                                op=ALU.mult)
                        recn = col.tile([P, 1], F32, tag="dprecn", name="dprecn")
                        nc.scalar.mul(out=recn, in_=rec, mul=-1.0)
                        s_cols.append(s_col)
                        recn_cols.append(recn)

                    dpyts = []
                    for m in range(NT):
                        szb = work.tile([P, C], F32, tag="scratch", name="szb")
                        nc.vector.tensor_scalar_mul(out=szb, in0=zb[m],
                                                    scalar1=s_cols[m])
                        yt = work.tile([P, C], F32, tag="scratch", name="dpy")

                        def cb(ps, m=m, szb=szb, yt=yt):
                            nc.vector.scalar_tensor_tensor(
                                out=yt, in0=ps, scalar=recn_cols[m], in1=szb,
                                op0=ALU.mult, op1=ALU.add)
                        B.matmul_nt(
                            [E[k][:, m * P:(m + 1) * P] for k in range(NT)],
                            zraw, cb)
                        dpyts.append(yt)
                    B.ln_batch(dpyts, dpout, dpg, dpbl)

                # ===== branch 1 (dp -> fa1/na1): overlaps with tc/bi =====
                if "fa" in phases and "dp" in phases:
                    _branch(nc, B, tcx, wpool, work, col, 1, dpout, wT_d,
                            b_d, g_ln_d, bln_d, na_wA_d, na_wp_d, na_b_d,
                            wv, pos_t, out_acc)

                # ===================== back_incor ====================
                if "bi" not in phases:
                    for m in range(NT):
                        nc.vector.memset(biout[m], 0.0)
                if "bi" in phases:
                  with tcx.tile_pool(name="bip", bufs=1) as bip:
                    Gt = [bip.tile([P, C], F32, tag=f"G{i}", name=f"G{i}")
                          for i in range(GT_TILES)]
                    nc.gpsimd.memset(Gt[GT_TILES - 1], 0.0)
                    for i in range(GT_TILES):
                        r0, r1 = i * P, min((i + 1) * P, GN)
                        nc.sync.dma_start(out=Gt[i][:r1 - r0, :],
                                          in_=g_d[r0:r1, :])
                    nfg = [bip.tile([P, C], F32, tag=f"nfg{i}", name=f"nfg{i}")
                           for i in range(GT_TILES)]
                    B.l2norm_rows(Gt, nfg)
                    GNP = GT_TILES * P
                    nfgT = [bip.tile([P, GNP], F32, tag=f"nfgT{k}",
                                     name=f"nfgT{k}") for k in range(CT)]
                    B.transpose_tiles(nfg, nfgT, rnd=True)

                    u_stack = [bip.tile([P, C], F32, tag=f"bu{m}",
                                        name=f"bu{m}") for m in range(NT)]
                    R = [bip.tile([P, M], F32, tag=f"bR{m}", name=f"bR{m}")
                         for m in range(NT)]
                    with tcx.tile_pool(name="gpp", bufs=3) as gpp:
                        for t in range(T):
                            mtile, moff = t // 4, (t % 4) * NPF
                            gpad = gpp.tile([M, C], F32, tag="gpad",
                                            name="gpad")
                            nc.sync.dma_start(out=gpad,
                                              in_=g_d[t * M:(t + 1) * M, :])
                            # Araw_t [32, 49] (raw, for concat path)
                            psA = pt.tile([P, M], F32, tag="pt", name="psA")
                            for k in range(CT):
                                B.mm(psA[:NPF, :],
                                     nfT[k][:, t * NPF:(t + 1) * NPF],
                                     nfgT[k][:, t * M:(t + 1) * M],
                                     start=(k == 0), stop=(k == CT - 1))
                            fs = col.tile([P, 1], F32, tag="bifs", name="bifs")
                            scr = work.tile([P, M], F32, tag="biscr",
                                            name="biscr")
                            nc.scalar.activation(out=scr[:NPF, :],
                                                 in_=psA[:NPF, :],
                                                 func=AF.Exp, scale=5.0,
                                                 accum_out=fs[:NPF, :])
                            nc.scalar.copy(out=R[mtile][moff:moff + NPF, :],
                                           in_=psA[:NPF, :])
                            rs = col.tile([P, 1], F32, tag="birs", name="birs")
                            nc.vector.reciprocal(out=rs[:NPF, :],
                                                 in_=fs[:NPF, :])
                            # ArawT_t [49, 32] -> FT = exp(5 ArawT)
                            psAT = pt.tile([P, NPF], F32, tag="pt", name="psAT")
                            for k in range(CT):
                                B.mm(psAT[:M, :],
                                     nfgT[k][:, t * M:(t + 1) * M],
                                     nfT[k][:, t * NPF:(t + 1) * NPF],
                                     start=(k == 0), stop=(k == CT - 1))
                            FT = work.tile([P, NPF], F32, tag="biFT",
                                           name="biFT")
                            fto = (FT[:M, :].bitcast(F32R) if B.use_f32r
                                   else FT[:M, :])
                            nc.scalar.activation(out=fto, in_=psAT[:M, :],
                                                 func=AF.Exp, scale=5.0)
                            gpr = gpp.tile([M, C], F32, tag="gpr",
                                           name="gpr")
                            B.rcopy(gpr, gpad)
                            # u_t = rs * (FT.T @ gf_t)
                            psu = pm.tile([P, C], F32, tag="pm", name="psu")
                            B.mm(psu[:NPF, :], FT[:M, :], gpr,
                                 start=True, stop=True)
                            nc.vector.tensor_scalar_mul(
                                out=u_stack[mtile][moff:moff + NPF, :],
                                in0=psu[:NPF, :], scalar1=rs[:NPF, :])

                    uT = [bip.tile([P, N], F32, tag=f"buT{k}", name=f"buT{k}")
                          for k in range(CT)]
                    B.transpose_tiles(u_stack, uT, rnd=True)
                    RT = bip.tile([M, N], F32, tag="bRT", name="bRT")
                    for m in range(NT):
                        ptr = pt.tile([P, P], F32, tag="pt", name="ptr")
                        nc.tensor.transpose(ptr[:M, :], R[m], ident)
                        B.rcopy(RT[:, m * P:(m + 1) * P], ptr[:M, :])

                    w1T = _load_w_tiles(nc, B, wpool, bi_w1T_d)
                    w2s = wpool.tile([M, C], F32, tag="w2s", name="w2s",
                                     bufs=1)
                    nc.sync.dma_start(out=w2s, in_=bi_w2T_d[:, :])
                    w2T = wpool.tile([M, C], F32, tag="w2", name="w2", bufs=1)
                    B.rcopy(w2T, w2s)
                    bib = _bcast_row(nc, bip, bi_b_d[:], C, tag="bib")
                    big_ = _bcast_row(nc, bip, bi_g_d[:], C, tag="big")
                    bibl = _bcast_row(nc, bip, bi_bln_d[:], C, tag="bibl")
                    biybs = []
                    for m in range(NT):
                        ps = pm.tile([P, C], F32, tag="pm", name="pm")
                        for k in range(CT):
                            B.mm(ps, uT[k][:, m * P:(m + 1) * P],
                                 w1T[k], start=(k == 0), stop=False)
                        B.mm(ps, RT[:, m * P:(m + 1) * P], w2T,
                             start=False, stop=True)
                        yb = work.tile([P, C], F32, tag="scratch", name="biyb")
                        nc.vector.tensor_tensor(out=yb, in0=ps, in1=bib,
                                                op=ALU.add)
                        biybs.append(yb)
                    B.ln_batch(biybs, biout, big_, bibl)

                # ===== branch 3 (bi -> fa3/na3): overlaps with tc =====
                if "fa" in phases and "bi" in phases:
                    _branch(nc, B, tcx, wpool, work, col, 3, biout, wT_d,
                            b_d, g_ln_d, bln_d, na_wA_d, na_wp_d, na_b_d,
                            wv, pos_t, out_acc)

                # ===================== temp_conv =====================
                if "tc" not in phases:
                    for m in range(NT):
                        nc.vector.memset(tcacc[m], 0.0)
                if "tc" in phases:
                  with tcx.tile_pool(name="tcp", bufs=1) as tcp:
                    BF16 = mybir.dt.bfloat16
                    # one-hot(argmax per frame) in bf16 (exact 0/1)
                    OH = [tcp.tile([P, N], BF16, tag=f"OH{m}", name=f"OH{m}")
                          for m in range(NT)]
                    for m in range(NT):
                        gmax = col.tile([P, T], F32, tag="gmax", name="gmax")
                        s3 = S_sb[m].rearrange("p (t n) -> p t n", t=T)
                        nc.vector.reduce_max(out=gmax, in_=s3, axis=AX.X)
                        g3 = gmax.rearrange("p (t o) -> p t o", o=1)
                        nc.vector.tensor_tensor(
                            out=OH[m].rearrange("p (t n) -> p t n", t=T),
                            in0=s3, in1=g3.broadcast_to((P, T, NPF)),
                            op=ALU.is_equal)
                    OHT = [tcp.tile([P, N], BF16, tag=f"OHT{m}",
                                    name=f"OHT{m}") for m in range(NT)]
                    for i in range(NT):
                        for j in range(NT):
                            ptb = pt.tile([P, P], BF16, tag="pt", name="ptb")
                            nc.tensor.transpose(
                                ptb, OH[j][:, i * P:(i + 1) * P], identb)
                            nc.vector.tensor_copy(
                                out=OHT[i][:, j * P:(j + 1) * P], in_=ptb)

                    tcg = _bcast_row(nc, tcp, tc_g_d[:], C, tag="tcg")
                    tcbl = _bcast_row(nc, tcp, tc_bln_d[:], C, tag="tcbl")

                    # Shift-packed operands: row r=(k,j) of column t holds
                    # frame (t+k): OH4/W4 for taps 0-3 (K=128), OH3/W3 for
                    # taps 4-6 (K=96). Repacking via SBUF->SBUF DMA.
                    OH4 = tcp.tile([P, T, N], BF16, tag="OH4", name="OH4")
                    OH3 = tcp.tile([97, T, N], BF16, tag="OH3", name="OH3")
                    W4 = tcp.tile([P, T, C], BF16, tag="W4", name="W4")
                    W3 = tcp.tile([97, T, C], BF16, tag="W3", name="W3")
                    # 97th row: ones x conv bias -> bias folds into the
                    # PSUM accumulation (one-hot rows each sum to 1)
                    nc.vector.memset(OH3[96:97, :, :], 1.0)
                    nc.sync.dma_start(
                        out=W3[96:97, :, :],
                        in_=bass.AP(tensor=tc_cbh_d[:].tensor,
                                    offset=tc_cbh_d[:].offset,
                                    ap=[[0, 1], [0, T], [1, C]]))
                    dmae = [nc.sync, nc.gpsimd]
                    for k in range(KW):
                        dstOH = OH4 if k < 4 else OH3
                        ro = 32 * k if k < 4 else 32 * (k - 4)
                        for t in range(T - k):
                            tp = t + k
                            tl, off = tp // 4, (tp % 4) * NPF
                            dmae[(k * T + t) % 2].dma_start(
                                out=dstOH[ro:ro + NPF, t, :],
                                in_=OHT[tl][off:off + NPF, :])
                    with tcx.tile_pool(name="xwp", bufs=2) as xwp:
                        for k in range(KW):
                            dstW = W4 if k < 4 else W3
                            ro = 32 * k if k < 4 else 32 * (k - 4)
                            cwk = _bcast_row(nc, xwp, tc_cw_d[k, :], C,
                                             tag="cwk")
                            Xwk = [xwp.tile([P, C], BF16, tag="xwk",
                                            name="xwk") for _ in range(NT)]
                            for mm in range(NT):
                                nc.vector.tensor_tensor(
                                    out=Xwk[mm], in0=X[mm], in1=cwk,
                                    op=ALU.mult)
                            for t in range(T - k):
                                tp = t + k
                                tl, off = tp // 4, (tp % 4) * NPF
                                dmae[(k * T + t) % 2].dma_start(
                                    out=dstW[ro:ro + NPF, t, :],
                                    in_=Xwk[tl][off:off + NPF, :])

                    for m in range(NT):
                        for to in range(TO):
                            ps = pm.tile([P, C], F32, tag="pm", name="pm")
                            nc.tensor.matmul(
                                ps, OH4[:, to, m * P:(m + 1) * P],
                                W4[:, to, :], start=True, stop=False)
                            nc.tensor.matmul(
                                ps, OH3[:, to, m * P:(m + 1) * P],
                                W3[:, to, :], start=False, stop=True)
                            # LN directly from PSUM; ACT-free inner loop
                            # except the [128,1] sqrt (single table)
                            stats = col.tile([P, 6], F32, tag="bnst",
                                             name="bnst")
                            nc.vector.bn_stats(out=stats, in_=ps)
                            mv = col.tile([P, 2], F32, tag="bnmv",
                                          name="bnmv")
                            nc.vector.bn_aggr(out=mv, in_=stats)
                            rstd = col.tile([P, 1], F32, tag="rstd",
                                            name="rstd")
                            nc.scalar.activation(out=rstd, in_=mv[:, 1:2],
                                                 func=AF.Sqrt, bias=B.eps_ln)
                            nc.vector.reciprocal(out=rstd, in_=rstd)
                            z1 = work.tile([P, C], F32, tag="scratch",
                                           name="tcz1")
                            nc.vector.scalar_tensor_tensor(
                                out=z1, in0=ps, scalar=mv[:, 0:1], in1=tcg,
                                op0=ALU.subtract, op1=ALU.mult)
                            zt = work.tile([P, C], F32, tag="scratch",
                                           name="tczt")
                            nc.vector.scalar_tensor_tensor(
                                out=zt, in0=z1, scalar=rstd, in1=tcbl,
                                op0=ALU.mult, op1=ALU.add)
                            z2 = work.tile([P, C], F32, tag="scratch",
                                           name="tcz2")
                            nc.vector.tensor_scalar_mul(out=z2, in0=zt,
                                                        scalar1=0.01)
                            if to == 0:
                                nc.vector.tensor_tensor(out=tcacc[m], in0=zt,
                                                        in1=z2, op=ALU.max)
                            else:
                                zl = work.tile([P, C], F32, tag="scratch",
                                               name="tczl")
                                nc.vector.tensor_tensor(out=zl, in0=zt,
                                                        in1=z2, op=ALU.max)
                                nc.vector.tensor_tensor(out=tcacc[m],
                                                        in0=tcacc[m], in1=zl,
                                                        op=ALU.add)
                        nc.scalar.mul(out=tcacc[m], in_=tcacc[m], mul=1.0 / TO)

            # ===== branch 2 (tc -> fa2/na2) =====
            if "fa" in phases and "tc" in phases:
                _branch(nc, B, tcx, wpool, work, col, 2, tcacc, wT_d,
                        b_d, g_ln_d, bln_d, na_wA_d, na_wp_d, na_b_d,
                        wv, pos_t, out_acc)

            for m in range(NT):
                nc.sync.dma_start(out=y_d[m * P:(m + 1) * P, :],
                                  in_=out_acc[m])

    split_multiwaits(nc)
    return nc



def _branch(nc, B, tcx, wpool, work, col, idx, xin, wT_d, b_d, g_ln_d,
            bln_d, na_wA_d, na_wp_d, na_b_d, wv, pos_t, out_acc):
    fa = f"fa{idx}"
    with tcx.tile_pool(name=f"br{idx}", bufs=1) as brp:
        faout = _feat_aggr(nc, B, brp, wpool, work, col, xin,
                           wT_d[fa], b_d[fa], g_ln_d[fa], bln_d[fa])
        _node_att(nc, B, brp, work, col, faout, pos_t,
                  na_wA_d[idx], na_wp_d[idx], na_b_d[idx],
                  wv[idx + 1], out_acc)


def _feat_aggr(nc, B, brp, wpool, work, col, xin, wT_dram, b_dram,
               g_dram, bln_dram):
    """y = lrelu(LN(softmax(5 nf nf^T) @ (x W^T + b)))"""
    xinT = [brp.tile([P, N], F32, tag=f"fxT{k}", name=f"fxT{k}")
            for k in range(CT)]
    B.transpose_tiles(xin, xinT, rnd=True)
    nfp = [brp.tile([P, C], F32, tag=f"fnf{m}", name=f"fnf{m}")
           for m in range(NT)]
    B.l2norm_rows(xin, nfp)
    nfpT = [brp.tile([P, N], F32, tag=f"fnfT{k}", name=f"fnfT{k}")
            for k in range(CT)]
    B.transpose_tiles(nfp, nfpT, rnd=True)

    wT = _load_w_tiles(nc, B, wpool, wT_dram)
    fab = _bcast_row(nc, brp, b_dram[:], C, tag="fab")
    fag = _bcast_row(nc, brp, g_dram[:], C, tag="fag")
    fabl = _bcast_row(nc, brp, bln_dram[:], C, tag="fabl")
    # bias moves out of the matmul: softmax rows sum to 1, so
    # A@(z + 1 b^T) = A@z + b  -> apply b after the aggregation.
    z = [brp.tile([P, C], F32, tag=f"fz{m}", name=f"fz{m}")
         for m in range(NT)]
    B.xw_plus(xinT, wT, out_raw=z)

    F = [brp.tile([P, N], F32, tag=f"fF{m}", name=f"fF{m}") for m in range(NT)]
    rs_cols = []
    for m in range(NT):
        fs = col.tile([P, 1], F32, tag="fafs", name="fafs")

        def cb(ps, m=m, fs=fs):
            fo = F[m].bitcast(F32R) if B.use_f32r else F[m]
            nc.scalar.activation(out=fo, in_=ps, func=AF.Exp, scale=5.0,
                                 accum_out=fs)
        B.matmul_nt([nfpT[k][:, m * P:(m + 1) * P] for k in range(CT)],
                    nfpT, cb, nfree=N)
        rs = col.tile([P, 1], F32, tag="fars", name="fars")
        nc.vector.reciprocal(out=rs, in_=fs)
        rs_cols.append(rs)

    out = [brp.tile([P, C], F32, tag=f"fo{m}", name=f"fo{m}")
           for m in range(NT)]
    yts = []
    for m in range(NT):
        yt = work.tile([P, C], F32, tag="scratch", name="fayt")

        def cb(ps, m=m, yt=yt):
            nc.vector.scalar_tensor_tensor(out=yt, in0=ps,
                                           scalar=rs_cols[m], in1=fab,
                                           op0=ALU.mult, op1=ALU.add)
        B.matmul_nt([F[k][:, m * P:(m + 1) * P] for k in range(NT)], z, cb)
        yts.append(yt)
    B.ln_batch(yts, out, fag, fabl)
    return out


def _node_att(nc, B, brp, work, col, xin, pos_t, wA_dram, wp_dram, b_dram,
              wv_col, out_acc):
    """out_acc += wv * (xin * sigmoid(nf@(nf^T@wA) + pos@wp + b))
    A@wA == nf @ (nf^T @ wA): two PE matvecs, A never materialized."""
    nfp = [brp.tile([P, C], F32, tag=f"nnf{m}", name=f"nnf{m}")
           for m in range(NT)]
    B.l2norm_rows(xin, nfp)
    nfpT = [brp.tile([P, N], F32, tag=f"nnfT{k}", name=f"nnfT{k}")
            for k in range(CT)]
    B.transpose_tiles(nfp, nfpT)

    wA = []
    for i in range(NT):
        t = brp.tile([P, 1], F32, tag=f"nwA{i}", name=f"nwA{i}")
        nc.sync.dma_start(out=t, in_=wA_dram[i * P:(i + 1) * P, :])
        wA.append(t)
    wp_row = _bcast_row(nc, brp, wp_dram[:], POSD, tag="nwp")
    b_row = _bcast_row(nc, brp, b_dram[:], 1, tag="nb")

    # v = nf^T @ wA (channel-major column, CT pieces [128, 1])
    v = []
    for c in range(CT):
        pv = B.pt.tile([P, 1], F32, tag="pt", name="pv")
        for m in range(NT):
            B.mm(pv, nfp[m][:, c * P:(c + 1) * P], wA[m],
                 start=(m == 0), stop=(m == NT - 1))
        vc = col.tile([P, 1], F32, tag="nav", name="nav")
        nc.scalar.copy(out=vc, in_=pv)
        v.append(vc)

    for m in range(NT):
        pa = B.pt.tile([P, 1], F32, tag="pt", name="pa")
        for k in range(CT):
            B.mm(pa, nfpT[k][:, m * P:(m + 1) * P], v[k],
                 start=(k == 0), stop=(k == CT - 1))
        sp = work.tile([P, POSD], F32, tag="naps", name="naps")
        nc.vector.tensor_tensor(out=sp, in0=pos_t[m], in1=wp_row, op=ALU.mult)
        pw = col.tile([P, 1], F32, tag="napw", name="napw")
        nc.vector.reduce_sum(out=pw, in_=sp, axis=AX.X)
        att = col.tile([P, 1], F32, tag="naatt", name="naatt")
        nc.vector.tensor_tensor(out=att, in0=pa, in1=pw, op=ALU.add)
        nc.scalar.activation(out=pw, in_=att, func=AF.Sigmoid,
                             bias=b_row, scale=1.0)
        contrib = work.tile([P, C], F32, tag="scratch", name="nact")
        nc.vector.tensor_scalar(out=contrib, in0=xin[m], scalar1=pw,
                                scalar2=wv_col, op0=ALU.mult, op1=ALU.mult)
        nc.gpsimd.tensor_tensor(out=out_acc[m], in0=out_acc[m], in1=contrib,
                                op=ALU.add)


# =====================================================================
# host wrapper
# =====================================================================

_NC_CACHE = {}


def _get_nc():
    if "nc" not in _NC_CACHE:
        _NC_CACHE["nc"] = build_nc()
    return _NC_CACHE["nc"]


def make_in_maps(local_feat, global_feat, pos, weights, params):
    def ap(v):
        return np.ascontiguousarray(np.asarray(v, dtype=np.float32))

    shared = {"wvec": ap(weights)}
    for o in ("dp", "fa1", "fa2", "fa3"):
        p = params[o]
        assert np.allclose(np.diagonal(ap(p["adj_w"])), 1.0), "adj diag != 1"
        shared[f"{o}_wT"] = ap(np.asarray(p["aff_w"]).T)
        shared[f"{o}_b"] = ap(p["aff_b"])
        shared[f"{o}_g"] = ap(p["ln_g"])
        shared[f"{o}_bln"] = ap(p["ln_b"])
    pbi = params["bi"]
    assert np.allclose(np.diagonal(ap(pbi["adj_w"])), 1.0)
    wbi = ap(pbi["aff_w"])          # [512, 561]
    shared["bi_w1T"] = ap(wbi[:, :C].T)
    shared["bi_w2T"] = ap(wbi[:, C:].T)
    shared["bi_b"] = ap(pbi["aff_b"])
    shared["bi_g"] = ap(pbi["ln_g"])
    shared["bi_bln"] = ap(pbi["ln_b"])
    ptc = params["tc"]
    assert np.allclose(np.diagonal(ap(ptc["adj_w"])), 1.0)
    import ml_dtypes
    shared["tc_cw"] = ap(np.asarray(ptc["conv_w"])[:, 0, :].T)   # [7, 512]
    shared["tc_cbh"] = np.ascontiguousarray(
        np.asarray(ptc["conv_b"], np.float32).astype(ml_dtypes.bfloat16))
    shared["tc_g"] = ap(ptc["ln_g"])
    shared["tc_bln"] = ap(ptc["ln_b"])
    for i, na in ((1, "na1"), (2, "na2"), (3, "na3")):
        att_w = ap(params[na]["att_w"])[0]      # [521]
        shared[f"na{i}_wA"] = ap(att_w[:N][:, None])
        shared[f"na{i}_wp"] = ap(att_w[N:])
        shared[f"na{i}_b"] = ap(params[na]["att_b"])

    in_maps = []
    for b in range(local_feat.shape[0]):
        m = dict(shared)
        m["x"] = ap(local_feat[b].reshape(N, C))
        m["g"] = ap(global_feat[b].reshape(GN, C))
        m["pos"] = ap(pos[b].reshape(N, POSD))
        in_maps.append(m)
    return in_maps


def kernel(local_feat, global_feat, pos, weights, params):
    from concourse.bass_utils import run_bass_kernel_spmd

    local_feat = np.asarray(local_feat, dtype=np.float32)
    global_feat = np.asarray(global_feat, dtype=np.float32)
    pos = np.asarray(pos, dtype=np.float32)
    weights = np.asarray(weights, dtype=np.float32)

    in_maps = make_in_maps(local_feat, global_feat, pos, weights, params)
    nc = _get_nc()
    res = run_bass_kernel_spmd(nc, in_maps, core_ids=list(range(8)))
    out = np.stack([res.results[b]["y"].reshape(T, NPF, C)
                    for b in range(8)])
    return out.astype(np.float32)
